# revision 16
# baseline (speedup 1.0000x reference)
"""LSH similarity-matrix kernel for Trainium2 (8 NeuronCores, data-parallel over batch).

Math: reference computes, per (l, b):
    c1 = (query_embed @ r.T > 0),  c2 = (doc_embed @ r.T > 0)   in {0,1}
    ham = s1 + s2 - 2*c1@c2.T ;  sim = cos(pi/NB * ham), masked where tok==0.
With +-1 codes U = 2c-1 and S = U1 @ U2.T:  ham = (NB - S)/2, so
    sim = sin(pi/(2*NB) * S).
Masks fold into the embeddings: a zeroed embedding row projects to 0,
sign(0) = 0 gives a zero code row, so S = 0 and sin(0) = 0 — exactly the
masked output. Masked doc tokens (half of them: tok in {0,1}) are gathered
away host-side entirely; output columns scatter back as zeros. Batches are
assigned to (core, slot) sorted by active-token count so every core runs an
identically-shaped program with minimal padding per slot.

Precision: the projection runs as a single float32r (TF32) matmul per
128-bit chunk. TF32's 11-bit mantissa flips ~1.4k of the 71M hash bits
(the ones whose fp32 projection sits within the rounding error of zero),
which perturbs the final similarity by rel err ~7e-3 end-to-end — well
inside the 2e-2 gate — at 1/3 the PE cost of a compensated projection.
The embeddings and r stream from HBM straight into float32r tiles (f32r
is an engine compute mode over fp32 bits, so the DMA is a plain byte
copy and no on-device cast is needed). The code dot runs as fp8e4m3
DoubleRow matmuls (chunk pairs give K=256 per MM at 0.5 cycles/row);
+-1/0 codes and their fp32 PSUM accumulation are exact.

The kernel is sign-throughput-bound (every projected bit crosses
PSUM->SBUF through DVE/ACT exactly once), so chunks are projected into
chunk-PAIR PSUM tiles and each sign instruction covers both chunks of a
pair (halving per-instruction access overhead); pairs alternate between
the DVE (clamp) and ACT (Sign) engines to balance their ns. r is
pre-scaled by 2^66 host-side so the DVE sign alternative
clamp(x, -1, 1) = max(min(x,1),-1) is exact (any |proj| > 2^-66 maps to
+-1). DMAs ride three independent queues (consts: ACT, doc loads: SP,
result stores: Pool SWDGE) so a store waiting on Sin can never
head-of-line block the next job's load.
"""
import os
import sys

sys.path.insert(0, "/opt/trn_rl_repo")

from contextlib import ExitStack

import numpy as np

import concourse.bass as bass
import concourse.mybir as mybir
import concourse.tile as tile
from concourse import bacc
from concourse.bass_utils import run_bass_kernel_spmd

L, BAT, A, BDOC, D, NB = 2, 32, 64, 1024, 128, 1024
CORES = 8
BPC = BAT // CORES          # batch slots per core
CH = NB // 128              # 8 bit-chunks
NPAIR = CH // 2             # chunk pairs per job
SCALE = float(2.0 ** 66)
PI = float(np.pi)

F32 = mybir.dt.float32
F32R = mybir.dt.float32r
BF16 = mybir.dt.bfloat16
FP8 = mybir.dt.float8e4
Alu = mybir.AluOpType
Act = mybir.ActivationFunctionType
DR = mybir.MatmulPerfMode.DoubleRow

_BUILD_CACHE: dict = {}

# doc-sign engine by chunk index: 1 = DVE clamp, 0 = ACT Sign.  ACT
# also carries the final Sin; DVE additionally takes all query chunks.
# Alternating 4/5 DVE chunks per job balances both engines' ns.
_DOC_DVE = ((1, 0, 1, 0, 1, 0, 1, 0),
            (1, 0, 1, 1, 0, 1, 0, 1))


def _col_splits(n):
    """Split [0, n) into equal-width pieces of <=512 columns (a matmul may
    not cross a PSUM bank, so piece i is written at PSUM column 512*i; a
    piece must also stay >=256 wide to keep f32r matmuls at full rate).
    Equal widths mean one strided access pattern covers all pieces, so
    sign/sin run as a single instruction per chunk pair. Returns
    (c0, c1, p0) per piece."""
    npieces = -(-n // 512)
    w = -(-(n // npieces) // 16) * 16
    while w * npieces < n:
        w += 16
    assert w * npieces >= n and w <= 512
    return [(i * w, min((i + 1) * w, n), 512 * i) for i in range(npieces)]


def _build(pads_c: tuple, qpad: int = A, reps: int = 1):
    """Per-core SPMD program. pads_c[b]: compute width (mult of 32) of batch
    slot b. reps > 1 re-emits the whole body (timing instrumentation only)."""
    pads_c = tuple(int(p) for p in pads_c)
    pad_cmax = max(pads_c)
    slot_splits = [_col_splits(p) for p in pads_c]
    np_max = max(len(s) for s in slot_splits)
    HW = np_max * 512              # PSUM columns per chunk tile

    nc = bacc.Bacc("TRN2", target_bir_lowering=False, debug=False)

    QW = BPC * L * qpad
    QE = nc.dram_tensor("qe", [D, QW], F32R, kind="ExternalInput").ap()
    DE = nc.dram_tensor("de", [BPC, L, D, pad_cmax], F32R,
                        kind="ExternalInput").ap()
    RT = nc.dram_tensor("rt", [D, NB], F32R, kind="ExternalInput").ap()
    OUT = nc.dram_tensor("out", [BPC, L, qpad, pad_cmax], F32, kind="ExternalOutput").ap()

    with tile.TileContext(nc) as tc, ExitStack() as ctx:
        const = ctx.enter_context(tc.tile_pool(name="const", bufs=1))
        jobp = ctx.enter_context(tc.tile_pool(name="jobp", bufs=2))
        outp = ctx.enter_context(tc.tile_pool(name="outp", bufs=2))
        # per-chunk psum tiles: 4 bufs x HW*4B = all 8 PSUM banks when
        # np_max == 2; 4-deep rotation decouples projections from signs
        ps_p = ctx.enter_context(tc.tile_pool(name="ps_p", bufs=4, space="PSUM"))

        for _rep in range(reps):
            _rp = f"r{_rep}_"
            # ---- constants: the first rt half leads the SP/HWDGE queue
            # (doc DMAs follow); qe + the rt tail ride the Pool SWDGE path,
            # which bypasses the single shared HWDGE dispatcher entirely ----
            rt = const.tile([D, NB], F32R, tag="rt", name=f"{_rp}rt")
            nc.sync.dma_start(out=rt[:, 0:512], in_=RT[:, 0:512])
            qnat = const.tile([D, QW], F32R, tag="qnat", name=f"{_rp}qnat")
            nc.gpsimd.dma_start(out=qnat, in_=QE)
            nc.gpsimd.dma_start(out=rt[:, 512:NB], in_=RT[:, 512:NB])

            # PE pre-warm: dependency-free dummy matmuls run while the first
            # DMAs land their completion receipts, pulling the PE through its
            # cold/mid clock ramp so the real projections start at 2.4 GHz.
            # warm's memset rides the otherwise-idle-at-t=0 DVE.
            warm = const.tile([D, 512], BF16, tag="warm", name=f"{_rp}warm")
            nc.vector.memset(warm, 0.0)
            wps = ps_p.tile([D, HW], F32, tag="pp",
                            name=f"{_rp}wps")[:, 0:512]
            for i in range(8):
                nc.tensor.matmul(wps, warm[:, 0:128], warm,
                                 start=True, stop=True)

            U1 = const.tile([D, CH * QW], FP8, tag="U1", name=f"{_rp}U1")

            def query_proj():
                for k in range(CH):
                    qp = ps_p.tile([D, HW], F32, tag="pp",
                                   name=f"{_rp}qp{k}")[:, 0:QW]
                    nc.tensor.matmul(qp, rt[:, k * 128:(k + 1) * 128], qnat,
                                     start=True, stop=True)
                    # query signs all ride DVE; ACT carries Sin + its higher
                    # per-instruction access cost
                    u1k = U1[:, k * QW:(k + 1) * QW]
                    nc.vector.tensor_scalar(u1k, qp, 1.0, -1.0,
                                            Alu.min, Alu.max)

            # ---- doc jobs, software-pipelined emission ----
            # stage A: dma;  stage B: project+sign;  stage C: code dot + sin
            # + dma out.  Emitting A(j+2)/B(j+1) before C(j) lets the PE run
            # projections while DVE/ACT finish the previous job's signs.
            _slot_order = sorted(range(BPC), key=lambda s: -pads_c[s])
            jobs = [(b, l) for b in _slot_order for l in range(L)]
            st = [dict() for _ in jobs]

            def stage_a(j):
                b, l = jobs[j]
                pad_c = pads_c[b]
                dnat = jobp.tile([D, pad_cmax], F32R, tag="dnat",
                                 name=f"{_rp}dnat{j}")[:, 0:pad_c]
                nc.sync.dma_start(out=dnat, in_=DE[b, l, :, 0:pad_c])
                st[j]["e"] = dnat

            def stage_b(j):
                b, l = jobs[j]
                pad_c = pads_c[b]
                splits = slot_splits[b]
                npieces = len(splits)
                w = splits[0][1] - splits[0][0]
                exact = npieces * w == pad_c
                ev = st[j]["e"]
                U2 = jobp.tile([D, CH * pad_cmax], FP8, tag="U2",
                               name=f"{_rp}U2{j}")
                for k in range(CH):
                    pp = ps_p.tile([D, HW], F32, tag="pp",
                                   name=f"{_rp}pp{j}_{k}")
                    rk = rt[:, k * 128:(k + 1) * 128]
                    for c0, c1, p0 in splits:
                        nc.tensor.matmul(pp[:, p0:p0 + c1 - c0], rk,
                                         ev[:, c0:c1], start=True, stop=True)
                    if exact:
                        if npieces == 1:
                            ppv = pp[:, 0:pad_c]
                            u2v = U2[:, k * pad_c:(k + 1) * pad_c]
                        else:
                            ppv = pp[:].rearrange("p (n c) -> p n c",
                                                  c=512)[:, 0:npieces, 0:w]
                            u2v = U2[:, k * pad_c:(k + 1) * pad_c] \
                                .rearrange("p (n c) -> p n c", c=w)
                        if _DOC_DVE[j % 2][k]:
                            nc.vector.tensor_scalar(u2v, ppv, 1.0, -1.0,
                                                    Alu.min, Alu.max)
                        else:
                            nc.scalar.activation(u2v, ppv, Act.Sign)
                    else:
                        for c0, c1, p0 in splits:
                            u2p = U2[:, k * pad_c + c0:k * pad_c + c1]
                            ppp = pp[:, p0:p0 + c1 - c0]
                            if _DOC_DVE[j % 2][k]:
                                nc.vector.tensor_scalar(u2p, ppp, 1.0, -1.0,
                                                        Alu.min, Alu.max)
                            else:
                                nc.scalar.activation(u2p, ppp, Act.Sign)
                st[j]["U2"] = U2

            def stage_c(j, split_tail=False):
                b, l = jobs[j]
                pad_c = pads_c[b]
                splits = slot_splits[b]
                npieces = len(splits)
                U2 = st[j]["U2"]
                # code dot via fp8 DoubleRow: chunk pairs (2jj, 2jj+1) fold
                # into one K=256 matmul; +-1/0 codes are exact in fp8e4m3
                S = ps_p.tile([qpad, HW], F32, tag="pp",
                              name=f"{_rp}S{j}")
                qcol = (b * L + l) * qpad
                sim = outp.tile([qpad, pad_cmax], F32, tag="sim",
                                name=f"{_rp}sim{j}")[:, 0:pad_c]
                w = splits[0][1] - splits[0][0]

                def dot(c0, c1, p0):
                    ww = c1 - c0
                    for jj in range(CH // 2):
                        lw = U1[:, 2 * jj * QW:(2 * jj + 2) * QW] \
                            .rearrange("p (o c) -> p o c", o=2)[:, :, qcol:qcol + qpad]
                        rv = U2[:, 2 * jj * pad_c:(2 * jj + 2) * pad_c] \
                            .rearrange("p (o c) -> p o c", o=2)[:, :, c0:c1]
                        nc.tensor.matmul(
                            S[:, p0:p0 + ww], lw, rv,
                            start=(jj == 0), stop=(jj == CH // 2 - 1),
                            perf_mode=DR,
                        )

                if split_tail:
                    # last job: per-piece dot->sin->store so the final store
                    # isn't serialized behind the full-width sin; halve
                    # single-piece jobs too
                    tsplits = splits
                    if npieces == 1:
                        hw2 = pad_c // 2
                        tsplits = [(0, hw2, 0), (hw2, pad_c, hw2)]
                    for pi, (c0, c1, p0) in enumerate(tsplits):
                        dot(c0, c1, p0)
                        nc.scalar.activation(sim[:, c0:c1], S[:, p0:p0 + c1 - c0],
                                             Act.Sin, scale=PI / (2.0 * NB))
                        eng = nc.sync if pi == len(tsplits) - 1 else nc.gpsimd
                        eng.dma_start(out=OUT[b, l, :, c0:c1], in_=sim[:, c0:c1])
                    return

                for c0, c1, p0 in splits:
                    dot(c0, c1, p0)
                if npieces > 1 and npieces * w == pad_c:
                    sv = S[:].rearrange("p (n c) -> p n c",
                                        c=512)[:, 0:npieces, 0:w]
                    mv = sim.rearrange("p (n c) -> p n c", c=w)
                    nc.scalar.activation(mv, sv, Act.Sin, scale=PI / (2.0 * NB))
                elif npieces == 1:
                    nc.scalar.activation(sim, S[:, 0:pad_c], Act.Sin,
                                         scale=PI / (2.0 * NB))
                else:
                    for c0, c1, p0 in splits:
                        nc.scalar.activation(sim[:, c0:c1], S[:, p0:p0 + c1 - c0],
                                             Act.Sin, scale=PI / (2.0 * NB))
                # result store on the Pool SWDGE queue (never blocks loads);
                # the very last store uses SP's lower-latency hwdge path
                eng = nc.sync if split_tail else nc.gpsimd
                eng.dma_start(out=OUT[b, l, :, 0:pad_c], in_=sim)

            n = len(jobs)
            stage_a(0)
            if n > 1:
                stage_a(1)
            stage_b(0)
            query_proj()
            for j in range(n - 1):
                if j + 2 < n:
                    stage_a(j + 2)
                stage_c(j)
                stage_b(j + 1)
            stage_c(n - 1, split_tail=True)

    nc.compile()
    return nc


def _stage_inputs(query_embed, doc_embed, query_tok, doc_tok, r):
    query_embed = np.ascontiguousarray(query_embed, dtype=np.float32)
    doc_embed = np.ascontiguousarray(doc_embed, dtype=np.float32)
    r = np.ascontiguousarray(r, dtype=np.float32)

    qmask = (np.asarray(query_tok) != 0)
    dmask = (np.asarray(doc_tok) != 0)

    # sort batches by active count; slot s takes ranks [s*CORES, (s+1)*CORES)
    # spread across the 8 cores, so per-slot padding is tight and identical
    # on every core (SPMD requires one shape per slot)
    counts = dmask.sum(axis=1).astype(int)
    order = np.argsort(counts, kind="stable")
    assign = np.empty((CORES, BPC), dtype=int)   # assign[c, b] = batch id
    for s in range(BPC):
        for c in range(CORES):
            assign[c, s] = order[s * CORES + c]
    pads_c = tuple(
        min(BDOC, max(64, int(-(-int(counts[assign[:, s]].max()) // 32) * 32)))
        for s in range(BPC)
    )
    pad_cmax = max(pads_c)

    qe_m = query_embed * qmask[None, :, :, None].astype(np.float32)
    qidxs = [np.flatnonzero(qmask[g]) for g in range(BAT)]
    qpad = min(A, max(16, int(-(-max(len(q) for q in qidxs) // 16) * 16)))
    QW = BPC * L * qpad

    rts = np.ascontiguousarray(r.T * SCALE)          # [D, NB], fp32 bits

    idxs = [np.flatnonzero(dmask[g]) for g in range(BAT)]
    in_maps = []
    for c in range(CORES):
        # embeddings staged pre-transposed [D, tokens]; queries compacted
        # to their active rows (masks are per-batch, shared by both layers)
        qe_c = np.zeros((D, QW), dtype=np.float32)
        de_c = np.zeros((BPC, L, D, pad_cmax), dtype=np.float32)
        for b in range(BPC):
            g = assign[c, b]
            qi = qidxs[g]
            for li in range(L):
                col = (b * L + li) * qpad
                qe_c[:, col:col + len(qi)] = qe_m[li, g, qi].T
            idx = idxs[g]
            de_c[b, :, :, :len(idx)] = doc_embed[:, g, idx].transpose(0, 2, 1)
        in_maps.append({"qe": qe_c, "de": de_c, "rt": rts})

    return in_maps, assign, idxs, pads_c, qidxs, qpad


def kernel(query_embed, doc_embed, query_tok, doc_tok, r):
    in_maps, assign, idxs, pads_c, qidxs, qpad = _stage_inputs(
        query_embed, doc_embed, query_tok, doc_tok, r)

    key = (pads_c, qpad)
    if key not in _BUILD_CACHE:
        _BUILD_CACHE[key] = _build(pads_c, qpad)
    nc = _BUILD_CACHE[key]

    res = run_bass_kernel_spmd(nc, in_maps, core_ids=list(range(CORES)))

    out = np.zeros((BAT, L, A, BDOC), dtype=np.float32)
    for c in range(CORES):
        o_c = res.results[c]["out"]  # [BPC, L, qpad, pad_cmax]
        for b in range(BPC):
            g = assign[c, b]
            idx = idxs[g]
            qi = qidxs[g]
            for li in range(L):
                out[g, li][np.ix_(qi, idx)] = o_c[b, li, :len(qi), :len(idx)]
    return out


# revision 23
# speedup vs baseline: 1.0237x; 1.0237x over previous
"""LSH similarity-matrix kernel for Trainium2 (8 NeuronCores, data-parallel over batch).

Math: reference computes, per (l, b):
    c1 = (query_embed @ r.T > 0),  c2 = (doc_embed @ r.T > 0)   in {0,1}
    ham = s1 + s2 - 2*c1@c2.T ;  sim = cos(pi/NB * ham), masked where tok==0.
With +-1 codes U = 2c-1 and S = U1 @ U2.T:  ham = (NB - S)/2, so
    sim = sin(pi/(2*NB) * S).
Masks fold into the embeddings: a zeroed embedding row projects to 0,
sign(0) = 0 gives a zero code row, so S = 0 and sin(0) = 0 — exactly the
masked output. Masked doc tokens (half of them: tok in {0,1}) are gathered
away host-side entirely; output columns scatter back as zeros. Batches are
assigned to (core, slot) sorted by active-token count so every core runs an
identically-shaped program with minimal padding per slot.

Precision: the projection runs as a single float32r (TF32) matmul per
128-bit chunk. TF32's 11-bit mantissa flips ~1.4k of the 71M hash bits
(the ones whose fp32 projection sits within the rounding error of zero),
which perturbs the final similarity by rel err ~7e-3 end-to-end — well
inside the 2e-2 gate — at 1/3 the PE cost of a compensated projection.
The embeddings and r stream from HBM straight into float32r tiles (f32r
is an engine compute mode over fp32 bits, so the DMA is a plain byte
copy and no on-device cast is needed). The code dot runs as fp8e4m3
DoubleRow matmuls (chunk pairs give K=256 per MM at 0.5 cycles/row);
+-1/0 codes and their fp32 PSUM accumulation are exact.

The kernel is sign-throughput-bound (every projected bit crosses
PSUM->SBUF through DVE/ACT exactly once), so chunks are projected into
chunk-PAIR PSUM tiles and each sign instruction covers both chunks of a
pair (halving per-instruction access overhead); pairs alternate between
the DVE (clamp) and ACT (Sign) engines to balance their ns. r is
pre-scaled by 2^66 host-side so the DVE sign alternative
clamp(x, -1, 1) = max(min(x,1),-1) is exact (any |proj| > 2^-66 maps to
+-1). DMAs ride three independent queues (consts: ACT, doc loads: SP,
result stores: Pool SWDGE) so a store waiting on Sin can never
head-of-line block the next job's load.
"""
import os
import sys

sys.path.insert(0, "/opt/trn_rl_repo")

from contextlib import ExitStack

import numpy as np

import concourse.bass as bass
import concourse.mybir as mybir
import concourse.tile as tile
from concourse import bacc
from concourse.bass_utils import run_bass_kernel_spmd

L, BAT, A, BDOC, D, NB = 2, 32, 64, 1024, 128, 1024
CORES = 8
BPC = BAT // CORES          # batch slots per core
CH = NB // 128              # 8 bit-chunks
NPAIR = CH // 2             # chunk pairs per job
SCALE = float(2.0 ** 66)
PI = float(np.pi)

F32 = mybir.dt.float32
F32R = mybir.dt.float32r
BF16 = mybir.dt.bfloat16
FP8 = mybir.dt.float8e4
Alu = mybir.AluOpType
Act = mybir.ActivationFunctionType
DR = mybir.MatmulPerfMode.DoubleRow

_BUILD_CACHE: dict = {}

# doc-sign engine by chunk index: 1 = DVE clamp, 0 = ACT Sign.  ACT
# also carries the final Sin; DVE additionally takes all query chunks.
# Alternating 4/5 DVE chunks per job balances both engines' ns.
_DOC_DVE = ((1, 0, 1, 0, 1, 0, 1, 0),
            (1, 0, 1, 1, 0, 1, 0, 1))


def _col_splits(n):
    """Split [0, n) into equal-width pieces of <=512 columns (a matmul may
    not cross a PSUM bank, so piece i is written at PSUM column 512*i; a
    piece must also stay >=256 wide to keep f32r matmuls at full rate).
    Equal widths mean one strided access pattern covers all pieces, so
    sign/sin run as a single instruction per chunk pair. Returns
    (c0, c1, p0) per piece."""
    npieces = -(-n // 512)
    w = -(-(n // npieces) // 16) * 16
    while w * npieces < n:
        w += 16
    assert w * npieces >= n and w <= 512
    return [(i * w, min((i + 1) * w, n), 512 * i) for i in range(npieces)]


def _build(pads_c: tuple, qpad: int = A, reps: int = 1):
    """Per-core SPMD program. pads_c[b]: compute width (mult of 32) of batch
    slot b. reps > 1 re-emits the whole body (timing instrumentation only)."""
    pads_c = tuple(int(p) for p in pads_c)
    pad_cmax = max(pads_c)
    slot_splits = [_col_splits(p) for p in pads_c]
    np_max = max(len(s) for s in slot_splits)
    HW = np_max * 512              # PSUM columns per chunk tile

    nc = bacc.Bacc("TRN2", target_bir_lowering=False, debug=False)

    QW = BPC * L * qpad
    QE = nc.dram_tensor("qe", [D, QW], F32R, kind="ExternalInput").ap()
    DE = nc.dram_tensor("de", [BPC, L, D, pad_cmax], F32R,
                        kind="ExternalInput").ap()
    RT = nc.dram_tensor("rt", [D, NB], F32R, kind="ExternalInput").ap()
    OUT = nc.dram_tensor("out", [BPC, L, qpad, pad_cmax], F32, kind="ExternalOutput").ap()

    with tile.TileContext(nc) as tc, ExitStack() as ctx:
        const = ctx.enter_context(tc.tile_pool(name="const", bufs=1))
        jobp = ctx.enter_context(tc.tile_pool(name="jobp", bufs=2))
        outp = ctx.enter_context(tc.tile_pool(name="outp", bufs=2))
        # per-chunk psum tiles: 4 bufs x HW*4B = all 8 PSUM banks when
        # np_max == 2; 4-deep rotation decouples projections from signs
        ps_p = ctx.enter_context(tc.tile_pool(name="ps_p", bufs=4, space="PSUM"))

        for _rep in range(reps):
            _rp = f"r{_rep}_"
            # ---- constants, ordered for the serialized DMA-transfer queue:
            # SP/HWDGE carries the rt pieces (chunk 0-1 weights first so the
            # first projection unblocks earliest); the Pool SWDGE path
            # (bypasses the shared HWDGE dispatcher) carries the first doc
            # load + qe ----
            rt = const.tile([D, NB], F32R, tag="rt", name=f"{_rp}rt")
            nc.sync.dma_start(out=rt[:, 0:256], in_=RT[:, 0:256])
            nc.sync.dma_start(out=rt[:, 256:512], in_=RT[:, 256:512])
            qnat = const.tile([D, QW], F32R, tag="qnat", name=f"{_rp}qnat")

            # PE pre-warm: dependency-free dummy matmuls run while the first
            # DMAs land their completion receipts, pulling the PE through its
            # cold/mid clock ramp so the real projections start at 2.4 GHz
            warm = const.tile([D, 512], BF16, tag="warm", name=f"{_rp}warm")
            nc.gpsimd.memset(warm, 0.0)
            wps = ps_p.tile([D, HW], F32, tag="pp",
                            name=f"{_rp}wps")[:, 0:512]
            for i in range(8):
                nc.tensor.matmul(wps, warm[:, 0:128], warm,
                                 start=True, stop=True)

            def load_consts_tail():
                nc.gpsimd.dma_start(out=qnat, in_=QE)
                nc.sync.dma_start(out=rt[:, 512:NB], in_=RT[:, 512:NB])

            U1 = const.tile([D, CH * QW], FP8, tag="U1", name=f"{_rp}U1")

            def query_proj():
                for k in range(CH):
                    qp = ps_p.tile([D, HW], F32, tag="pp",
                                   name=f"{_rp}qp{k}")[:, 0:QW]
                    nc.tensor.matmul(qp, rt[:, k * 128:(k + 1) * 128], qnat,
                                     start=True, stop=True)
                    # query signs all ride DVE; ACT carries Sin + its higher
                    # per-instruction access cost
                    u1k = U1[:, k * QW:(k + 1) * QW]
                    nc.vector.tensor_scalar(u1k, qp, 1.0, -1.0,
                                            Alu.min, Alu.max)

            # ---- doc jobs, software-pipelined emission ----
            # stage A: dma;  stage B: project+sign;  stage C: code dot + sin
            # + dma out.  Emitting A(j+2)/B(j+1) before C(j) lets the PE run
            # projections while DVE/ACT finish the previous job's signs.
            _slot_order = sorted(range(BPC), key=lambda s: -pads_c[s])
            jobs = [(b, l) for b in _slot_order for l in range(L)]
            st = [dict() for _ in jobs]

            def stage_a(j):
                b, l = jobs[j]
                pad_c = pads_c[b]
                dnat = jobp.tile([D, pad_cmax], F32R, tag="dnat",
                                 name=f"{_rp}dnat{j}")[:, 0:pad_c]
                # job 0's load takes the Pool SWDGE path: it runs concurrently
                # with the rt pieces on SP, landing ~1.3us earlier
                eng = nc.gpsimd if j == 0 else nc.sync
                eng.dma_start(out=dnat, in_=DE[b, l, :, 0:pad_c])
                st[j]["e"] = dnat

            def stage_b(j):
                b, l = jobs[j]
                pad_c = pads_c[b]
                splits = slot_splits[b]
                npieces = len(splits)
                w = splits[0][1] - splits[0][0]
                exact = npieces * w == pad_c
                ev = st[j]["e"]
                U2 = jobp.tile([D, CH * pad_cmax], FP8, tag="U2",
                               name=f"{_rp}U2{j}")
                for k in range(CH):
                    pp = ps_p.tile([D, HW], F32, tag="pp",
                                   name=f"{_rp}pp{j}_{k}")
                    rk = rt[:, k * 128:(k + 1) * 128]
                    for c0, c1, p0 in splits:
                        nc.tensor.matmul(pp[:, p0:p0 + c1 - c0], rk,
                                         ev[:, c0:c1], start=True, stop=True)
                    if exact:
                        if npieces == 1:
                            ppv = pp[:, 0:pad_c]
                            u2v = U2[:, k * pad_c:(k + 1) * pad_c]
                        else:
                            ppv = pp[:].rearrange("p (n c) -> p n c",
                                                  c=512)[:, 0:npieces, 0:w]
                            u2v = U2[:, k * pad_c:(k + 1) * pad_c] \
                                .rearrange("p (n c) -> p n c", c=w)
                        if _DOC_DVE[j % 2][k]:
                            nc.vector.tensor_scalar(u2v, ppv, 1.0, -1.0,
                                                    Alu.min, Alu.max)
                        else:
                            nc.scalar.activation(u2v, ppv, Act.Sign)
                    else:
                        for c0, c1, p0 in splits:
                            u2p = U2[:, k * pad_c + c0:k * pad_c + c1]
                            ppp = pp[:, p0:p0 + c1 - c0]
                            if _DOC_DVE[j % 2][k]:
                                nc.vector.tensor_scalar(u2p, ppp, 1.0, -1.0,
                                                        Alu.min, Alu.max)
                            else:
                                nc.scalar.activation(u2p, ppp, Act.Sign)
                st[j]["U2"] = U2

            def stage_c(j, split_tail=False):
                b, l = jobs[j]
                pad_c = pads_c[b]
                splits = slot_splits[b]
                npieces = len(splits)
                U2 = st[j]["U2"]
                # code dot via fp8 DoubleRow: chunk pairs (2jj, 2jj+1) fold
                # into one K=256 matmul; +-1/0 codes are exact in fp8e4m3
                S = None
                qcol = (b * L + l) * qpad
                sim = outp.tile([qpad, pad_cmax], F32, tag="sim",
                                name=f"{_rp}sim{j}")[:, 0:pad_c]
                w = splits[0][1] - splits[0][0]

                def dot(c0, c1, p0):
                    ww = c1 - c0
                    for jj in range(CH // 2):
                        lw = U1[:, 2 * jj * QW:(2 * jj + 2) * QW] \
                            .rearrange("p (o c) -> p o c", o=2)[:, :, qcol:qcol + qpad]
                        rv = U2[:, 2 * jj * pad_c:(2 * jj + 2) * pad_c] \
                            .rearrange("p (o c) -> p o c", o=2)[:, :, c0:c1]
                        nc.tensor.matmul(
                            S[:, p0:p0 + ww], lw, rv,
                            start=(jj == 0), stop=(jj == CH // 2 - 1),
                            perf_mode=DR,
                        )

                if split_tail:
                    # last job: per-piece dot->sin->store with a separate S
                    # tile per piece (a shared tile would serialize piece 2's
                    # dot behind piece 1's sin read); halve single-piece jobs
                    tsplits = splits
                    if npieces == 1:
                        hw2 = pad_c // 2
                        tsplits = [(0, hw2, 0), (hw2, pad_c, hw2)]
                    for pi, (c0, c1, p0) in enumerate(tsplits):
                        S = ps_p.tile([qpad, HW], F32, tag="pp",
                                      name=f"{_rp}St{j}_{pi}")
                        dot(c0, c1, 0)
                        nc.scalar.activation(sim[:, c0:c1], S[:, 0:c1 - c0],
                                             Act.Sin, scale=PI / (2.0 * NB))
                        eng = nc.sync if pi == len(tsplits) - 1 else nc.gpsimd
                        eng.dma_start(out=OUT[b, l, :, c0:c1], in_=sim[:, c0:c1])
                    return

                S = ps_p.tile([qpad, HW], F32, tag="pp", name=f"{_rp}S{j}")
                for c0, c1, p0 in splits:
                    dot(c0, c1, p0)
                if npieces > 1 and npieces * w == pad_c:
                    sv = S[:].rearrange("p (n c) -> p n c",
                                        c=512)[:, 0:npieces, 0:w]
                    mv = sim.rearrange("p (n c) -> p n c", c=w)
                    nc.scalar.activation(mv, sv, Act.Sin, scale=PI / (2.0 * NB))
                elif npieces == 1:
                    nc.scalar.activation(sim, S[:, 0:pad_c], Act.Sin,
                                         scale=PI / (2.0 * NB))
                else:
                    for c0, c1, p0 in splits:
                        nc.scalar.activation(sim[:, c0:c1], S[:, p0:p0 + c1 - c0],
                                             Act.Sin, scale=PI / (2.0 * NB))
                # result store on the Pool SWDGE queue (never blocks loads);
                # the very last store uses SP's lower-latency hwdge path
                eng = nc.sync if split_tail else nc.gpsimd
                eng.dma_start(out=OUT[b, l, :, 0:pad_c], in_=sim)

            n = len(jobs)
            stage_a(0)
            load_consts_tail()
            if n > 1:
                stage_a(1)
            stage_b(0)
            query_proj()
            for j in range(n - 1):
                if j + 2 < n:
                    stage_a(j + 2)
                stage_c(j)
                stage_b(j + 1)
            stage_c(n - 1, split_tail=True)

    nc.compile()
    return nc


def _stage_inputs(query_embed, doc_embed, query_tok, doc_tok, r):
    query_embed = np.ascontiguousarray(query_embed, dtype=np.float32)
    doc_embed = np.ascontiguousarray(doc_embed, dtype=np.float32)
    r = np.ascontiguousarray(r, dtype=np.float32)

    qmask = (np.asarray(query_tok) != 0)
    dmask = (np.asarray(doc_tok) != 0)

    # sort batches by active count; slot s takes ranks [s*CORES, (s+1)*CORES)
    # spread across the 8 cores, so per-slot padding is tight and identical
    # on every core (SPMD requires one shape per slot)
    counts = dmask.sum(axis=1).astype(int)
    order = np.argsort(counts, kind="stable")
    assign = np.empty((CORES, BPC), dtype=int)   # assign[c, b] = batch id
    for s in range(BPC):
        for c in range(CORES):
            assign[c, s] = order[s * CORES + c]
    pads_c = tuple(
        min(BDOC, max(64, int(-(-int(counts[assign[:, s]].max()) // 32) * 32)))
        for s in range(BPC)
    )
    pad_cmax = max(pads_c)

    qe_m = query_embed * qmask[None, :, :, None].astype(np.float32)
    qidxs = [np.flatnonzero(qmask[g]) for g in range(BAT)]
    qpad = min(A, max(16, int(-(-max(len(q) for q in qidxs) // 16) * 16)))
    QW = BPC * L * qpad

    rts = np.ascontiguousarray(r.T * SCALE)          # [D, NB], fp32 bits

    idxs = [np.flatnonzero(dmask[g]) for g in range(BAT)]
    in_maps = []
    for c in range(CORES):
        # embeddings staged pre-transposed [D, tokens]; queries compacted
        # to their active rows (masks are per-batch, shared by both layers)
        qe_c = np.zeros((D, QW), dtype=np.float32)
        de_c = np.zeros((BPC, L, D, pad_cmax), dtype=np.float32)
        for b in range(BPC):
            g = assign[c, b]
            qi = qidxs[g]
            for li in range(L):
                col = (b * L + li) * qpad
                qe_c[:, col:col + len(qi)] = qe_m[li, g, qi].T
            idx = idxs[g]
            de_c[b, :, :, :len(idx)] = doc_embed[:, g, idx].transpose(0, 2, 1)
        in_maps.append({"qe": qe_c, "de": de_c, "rt": rts})

    return in_maps, assign, idxs, pads_c, qidxs, qpad


def kernel(query_embed, doc_embed, query_tok, doc_tok, r):
    in_maps, assign, idxs, pads_c, qidxs, qpad = _stage_inputs(
        query_embed, doc_embed, query_tok, doc_tok, r)

    key = (pads_c, qpad)
    if key not in _BUILD_CACHE:
        _BUILD_CACHE[key] = _build(pads_c, qpad)
    nc = _BUILD_CACHE[key]

    res = run_bass_kernel_spmd(nc, in_maps, core_ids=list(range(CORES)))

    out = np.zeros((BAT, L, A, BDOC), dtype=np.float32)
    for c in range(CORES):
        o_c = res.results[c]["out"]  # [BPC, L, qpad, pad_cmax]
        for b in range(BPC):
            g = assign[c, b]
            idx = idxs[g]
            qi = qidxs[g]
            for li in range(L):
                out[g, li][np.ix_(qi, idx)] = o_c[b, li, :len(qi), :len(idx)]
    return out


# revision 24
# speedup vs baseline: 1.1271x; 1.1010x over previous
"""LSH similarity-matrix kernel for Trainium2 (8 NeuronCores, data-parallel over batch).

Math: reference computes, per (l, b):
    c1 = (query_embed @ r.T > 0),  c2 = (doc_embed @ r.T > 0)   in {0,1}
    ham = s1 + s2 - 2*c1@c2.T ;  sim = cos(pi/NB * ham), masked where tok==0.
With +-1 codes U = 2c-1 and S = U1 @ U2.T:  ham = (NB - S)/2, so
    sim = sin(pi/(2*NB) * S).
Masks fold into the embeddings: a zeroed embedding row projects to 0,
sign(0) = 0 gives a zero code row, so S = 0 and sin(0) = 0 — exactly the
masked output. Masked doc tokens (half of them: tok in {0,1}) are gathered
away host-side entirely; output columns scatter back as zeros.

Sharding: batches are assigned to (core, slot) sorted by active-token
count; every slot is capped at 512 docs so each chunk's projection fits a
single PSUM bank. Docs beyond 512 (a few tens per heavy batch) form one
extra "overflow" job per core: fixed per-slot segments, both layers side
by side, so the SPMD program shape is identical on every core.

Precision: the projection runs as a single float32r (TF32) matmul per
128-bit chunk. TF32's 11-bit mantissa flips ~1.4k of the 71M hash bits
(those whose fp32 projection sits within rounding error of zero), which
perturbs the final similarity by rel err ~7e-3 end-to-end — well inside
the 2e-2 gate — at 1/3 the PE cost of a compensated projection. The
embeddings and r stream from HBM straight into float32r tiles (f32r is
an engine compute mode over fp32 bits, so the DMA is a plain byte copy
and no on-device cast is needed). The code dot runs as fp8e4m3 DoubleRow
matmuls (chunk pairs give K=256 per MM at 0.5 cycles/row); +-1/0 codes
and their fp32 PSUM accumulation are exact.

The kernel is sign-throughput-bound: every projected bit crosses
PSUM->SBUF through DVE/ACT exactly once. So chunks are projected in
PAIRS into one 2-bank PSUM tile and signed by a single instruction
(halving per-instruction access overhead); pairs alternate between the
DVE (clamp) and ACT (Sign) engines, weighted so both engines carry equal
ns; query pairs interleave into the first two jobs' slack. r is
pre-scaled by 2^66 host-side so the DVE clamp(x,-1,1) = max(min(x,1),-1)
sign is exact (any |proj| > 2^-66 maps to +-1). DMAs ride independent
queues (loads: SP/HWDGE, stores: Pool SWDGE, which also bypasses the
shared HWDGE dispatcher for the first doc load) so a store waiting on
Sin never blocks the next job's load.
"""
import os
import sys

sys.path.insert(0, "/opt/trn_rl_repo")

from contextlib import ExitStack

import numpy as np

import concourse.bass as bass
import concourse.mybir as mybir
import concourse.tile as tile
from concourse import bacc
from concourse.bass_utils import run_bass_kernel_spmd

L, BAT, A, BDOC, D, NB = 2, 32, 64, 1024, 128, 1024
CORES = 8
BPC = BAT // CORES          # batch slots per core
CH = NB // 128              # 8 bit-chunks
NPAIR = CH // 2             # chunk pairs
CAP = 512                   # per-slot doc cap (one PSUM bank)
SCALE = float(2.0 ** 66)
PI = float(np.pi)

F32 = mybir.dt.float32
F32R = mybir.dt.float32r
BF16 = mybir.dt.bfloat16
FP8 = mybir.dt.float8e4
Alu = mybir.AluOpType
Act = mybir.ActivationFunctionType
DR = mybir.MatmulPerfMode.DoubleRow

_BUILD_CACHE: dict = {}

# chunk-pair sign engine: 1 = DVE clamp, 0 = ACT Sign (ACT also runs Sin)
_PAIR_DVE = (1, 0, 1, 0)
_QPAIR_DVE = (1, 0, 1, 1)   # 3 DVE / 1 ACT balances ACT's Sin load


def _build(pads_c: tuple, qpad: int, seg_os: tuple, reps: int = 1):
    """Per-core SPMD program. pads_c[b] <= CAP: width of main slot b;
    seg_os[b]: overflow segment width of slot b (0 = none). reps > 1
    re-emits the whole body (timing instrumentation only)."""
    pads_c = tuple(int(p) for p in pads_c)
    seg_os = tuple(int(s) for s in seg_os)
    pad_cmax = max(pads_c)
    OV = sum(seg_os)
    seg_off = [sum(seg_os[:s]) for s in range(BPC)]
    OV2 = 2 * OV                    # both layers side by side
    assert OV2 <= 512, "overflow exceeds one PSUM bank"

    nc = bacc.Bacc("TRN2", target_bir_lowering=False, debug=False)

    QW = BPC * L * qpad
    QE = nc.dram_tensor("qe", [D, QW], F32R, kind="ExternalInput").ap()
    DE = nc.dram_tensor("de", [BPC, L, D, pad_cmax], F32R,
                        kind="ExternalInput").ap()
    RT = nc.dram_tensor("rt", [D, NB], F32R, kind="ExternalInput").ap()
    OUT = nc.dram_tensor("out", [BPC, L, qpad, pad_cmax], F32,
                         kind="ExternalOutput").ap()
    if OV:
        DOV = nc.dram_tensor("dov", [D, OV2], F32R, kind="ExternalInput").ap()
        OUT2 = nc.dram_tensor("out2", [qpad, OV2], F32,
                              kind="ExternalOutput").ap()

    with tile.TileContext(nc) as tc, ExitStack() as ctx:
        const = ctx.enter_context(tc.tile_pool(name="const", bufs=1))
        jobp = ctx.enter_context(tc.tile_pool(name="jobp", bufs=3))
        outp = ctx.enter_context(tc.tile_pool(name="outp", bufs=2))
        # PSUM: chunk-pair tiles [128, 1024] (2 banks) x 3 bufs, plus the
        # dot-output tiles [*, 512] (1 bank) x 2 bufs = all 8 banks.
        ps_p = ctx.enter_context(tc.tile_pool(name="ps_p", bufs=3, space="PSUM"))

        for _rep in range(reps):
            _rp = f"r{_rep}_"
            # ---- constants, ordered for the serialized DMA-transfer queue:
            # SP/HWDGE carries the rt pieces (chunk 0-1 weights first so the
            # first projection unblocks earliest); the Pool SWDGE path
            # (bypasses the shared HWDGE dispatcher) carries the first doc
            # load + qe ----
            rt = const.tile([D, NB], F32R, tag="rt", name=f"{_rp}rt")
            nc.sync.dma_start(out=rt[:, 0:256], in_=RT[:, 0:256])
            nc.sync.dma_start(out=rt[:, 256:512], in_=RT[:, 256:512])
            qnat = const.tile([D, QW], F32R, tag="qnat", name=f"{_rp}qnat")

            # PE pre-warm: dependency-free dummy matmuls run while the first
            # DMAs land their completion receipts, pulling the PE through its
            # cold/mid clock ramp so the real projections start at 2.4 GHz
            warm = const.tile([D, 512], BF16, tag="warm", name=f"{_rp}warm")
            nc.gpsimd.memset(warm, 0.0)
            wps = ps_p.tile([D, 1024], F32, tag="pp",
                            name=f"{_rp}wps")[:, 0:512]
            for i in range(8):
                nc.tensor.matmul(wps, warm[:, 0:128], warm,
                                 start=True, stop=True)

            def load_consts_tail():
                nc.gpsimd.dma_start(out=qnat, in_=QE)
                nc.sync.dma_start(out=rt[:, 512:NB], in_=RT[:, 512:NB])

            U1 = const.tile([D, CH * QW], FP8, tag="U1", name=f"{_rp}U1")

            def _pair_sign(pp, dst2, wcols, dve):
                """One instruction signs both chunks of a pair: pp cols
                [0:w] and [512:512+w] -> dst2 [p, 2, w]."""
                if wcols == 512:
                    sv = pp[:]                                  # [p, 1024]
                    dv = dst2
                else:
                    sv = pp[:].rearrange("p (h c) -> p h c",
                                         h=2)[:, :, 0:wcols]
                    dv = dst2.rearrange("p (h c) -> p h c", h=2)
                if dve:
                    nc.vector.tensor_scalar(dv, sv, 1.0, -1.0,
                                            Alu.min, Alu.max)
                else:
                    nc.scalar.activation(dv, sv, Act.Sign)

            def query_grp(g):
                # query chunk pairs 2g, 2g+1; emitted inside the first two
                # jobs so the sign work fills both engines' slack
                for pr in (2 * g, 2 * g + 1):
                    qp = ps_p.tile([D, 1024], F32, tag="pp",
                                   name=f"{_rp}qp{pr}")
                    for h in (0, 1):
                        k = 2 * pr + h
                        nc.tensor.matmul(qp[:, h * 512:h * 512 + QW],
                                         rt[:, k * 128:(k + 1) * 128], qnat,
                                         start=True, stop=True)
                    _pair_sign(qp, U1[:, 2 * pr * QW:(2 * pr + 2) * QW],
                               QW, _QPAIR_DVE[pr])

            # ---- doc jobs, software-pipelined emission ----
            jobs = [(b, l) for b in range(BPC) for l in range(L)]
            n = len(jobs)
            st = [dict() for _ in range(n + 1)]     # [-1] = overflow job

            def stage_a(j):
                b, l = jobs[j]
                pad_c = pads_c[b]
                dnat = jobp.tile([D, pad_cmax], F32R, tag="dnat",
                                 name=f"{_rp}dnat{j}")[:, 0:pad_c]
                # job 0's load takes the Pool SWDGE path: it runs
                # concurrently with the rt pieces on SP
                eng = nc.gpsimd if j == 0 else nc.sync
                eng.dma_start(out=dnat, in_=DE[b, l, :, 0:pad_c])
                st[j]["e"] = dnat

            def stage_b(j):
                b, l = jobs[j]
                pad_c = pads_c[b]
                ev = st[j]["e"]
                U2 = jobp.tile([D, CH * pad_cmax], FP8, tag="U2",
                               name=f"{_rp}U2{j}")
                for pr in range(NPAIR):
                    pp = ps_p.tile([D, 1024], F32, tag="pp",
                                   name=f"{_rp}pp{j}_{pr}")
                    for h in (0, 1):
                        k = 2 * pr + h
                        nc.tensor.matmul(pp[:, h * 512:h * 512 + pad_c],
                                         rt[:, k * 128:(k + 1) * 128], ev,
                                         start=True, stop=True)
                    _pair_sign(pp, U2[:, 2 * pr * pad_c:(2 * pr + 2) * pad_c],
                               pad_c, _PAIR_DVE[pr])
                st[j]["U2"] = U2

            def _dot(S, U2, pad_c, qcol, c0, c1, p0):
                for jj in range(NPAIR):
                    lw = U1[:, 2 * jj * QW:(2 * jj + 2) * QW] \
                        .rearrange("p (o c) -> p o c", o=2)[:, :, qcol:qcol + qpad]
                    rv = U2[:, 2 * jj * pad_c:(2 * jj + 2) * pad_c] \
                        .rearrange("p (o c) -> p o c", o=2)[:, :, c0:c1]
                    nc.tensor.matmul(S[:, p0:p0 + c1 - c0], lw, rv,
                                     start=(jj == 0), stop=(jj == NPAIR - 1),
                                     perf_mode=DR)

            def stage_c(j, tail=False):
                b, l = jobs[j]
                pad_c = pads_c[b]
                U2 = st[j]["U2"]
                qcol = (b * L + l) * qpad
                sim = outp.tile([qpad, pad_cmax], F32, tag="sim",
                                name=f"{_rp}sim{j}")[:, 0:pad_c]
                if tail:
                    # last job (no overflow): per-half dot->sin->store with
                    # separate S tiles so the final store isn't serialized
                    # behind the full-width sin
                    hw2 = pad_c // 2
                    for pi, (c0, c1) in enumerate([(0, hw2), (hw2, pad_c)]):
                        S = ps_p.tile([qpad, 512], F32, tag="s",
                                      bufs=2, name=f"{_rp}St{pi}")
                        _dot(S, U2, pad_c, qcol, c0, c1, 0)
                        nc.scalar.activation(sim[:, c0:c1], S[:, 0:c1 - c0],
                                             Act.Sin, scale=PI / (2.0 * NB))
                        eng = nc.sync if pi else nc.gpsimd
                        eng.dma_start(out=OUT[b, l, :, c0:c1],
                                      in_=sim[:, c0:c1])
                    return
                S = ps_p.tile([qpad, 512], F32, tag="s", bufs=2,
                              name=f"{_rp}S{j}")
                _dot(S, U2, pad_c, qcol, 0, pad_c, 0)
                nc.scalar.activation(sim, S[:, 0:pad_c], Act.Sin,
                                     scale=PI / (2.0 * NB))
                # stores ride the Pool SWDGE queue: never block loads
                nc.gpsimd.dma_start(out=OUT[b, l, :, 0:pad_c], in_=sim)

            # ---- overflow job: one combined-layer job; segment s holds the
            # docs of slot s beyond CAP, layers side by side ----
            def stage_a_ov():
                dnat = jobp.tile([D, pad_cmax], F32R, tag="dnat",
                                 name=f"{_rp}dnatov")[:, 0:OV2]
                nc.sync.dma_start(out=dnat, in_=DOV)
                st[n]["e"] = dnat

            def stage_b_ov():
                ev = st[n]["e"]
                U2 = jobp.tile([D, CH * pad_cmax], FP8, tag="U2",
                               name=f"{_rp}U2ov")
                for pr in range(NPAIR):
                    pp = ps_p.tile([D, 1024], F32, tag="pp",
                                   name=f"{_rp}ppov{pr}")
                    for h in (0, 1):
                        k = 2 * pr + h
                        nc.tensor.matmul(pp[:, h * 512:h * 512 + OV2],
                                         rt[:, k * 128:(k + 1) * 128], ev,
                                         start=True, stop=True)
                    _pair_sign(pp, U2[:, 2 * pr * OV2:(2 * pr + 2) * OV2],
                               OV2, _PAIR_DVE[pr])
                st[n]["U2"] = U2

            def stage_c_ov():
                U2 = st[n]["U2"]
                S = ps_p.tile([qpad, 512], F32, tag="s", bufs=2,
                              name=f"{_rp}Sov")
                sim = outp.tile([qpad, pad_cmax], F32, tag="sim",
                                name=f"{_rp}simov")[:, 0:OV2]
                for li in range(L):
                    for s in range(BPC):
                        if not seg_os[s]:
                            continue
                        c0 = li * OV + seg_off[s]
                        qcol = (s * L + li) * qpad
                        _dot(S, U2, OV2, qcol, c0, c0 + seg_os[s], c0)
                nc.scalar.activation(sim, S[:, 0:OV2], Act.Sin,
                                     scale=PI / (2.0 * NB))
                nc.sync.dma_start(out=OUT2, in_=sim)

            stage_a(0)
            load_consts_tail()
            stage_a(1)
            stage_b(0)
            query_grp(0)
            stage_a(2)
            stage_b(1)
            query_grp(1)
            stage_a(3)
            for j in range(n):
                if j + 2 < n:
                    stage_b(j + 2)
                elif OV and j + 2 == n:
                    stage_b_ov()
                stage_c(j, tail=(not OV and j == n - 1))
                if j + 4 < n:
                    stage_a(j + 4)
                elif OV and j + 4 == n:
                    stage_a_ov()
            if OV:
                stage_c_ov()

    nc.compile()
    return nc


def _stage_inputs(query_embed, doc_embed, query_tok, doc_tok, r):
    query_embed = np.ascontiguousarray(query_embed, dtype=np.float32)
    doc_embed = np.ascontiguousarray(doc_embed, dtype=np.float32)
    r = np.ascontiguousarray(r, dtype=np.float32)

    qmask = (np.asarray(query_tok) != 0)
    dmask = (np.asarray(doc_tok) != 0)

    # sort batches by active count; slot s takes ranks [s*CORES, (s+1)*CORES)
    # spread across the 8 cores, so per-slot padding is tight and identical
    # on every core (SPMD requires one shape per slot)
    counts = dmask.sum(axis=1).astype(int)
    order = np.argsort(counts, kind="stable")
    assign = np.empty((CORES, BPC), dtype=int)   # assign[c, b] = batch id
    for s in range(BPC):
        for c in range(CORES):
            assign[c, s] = order[s * CORES + c]
    maxes = [int(counts[assign[:, s]].max()) for s in range(BPC)]
    pads_c = tuple(min(CAP, max(64, -(-m // 32) * 32)) for m in maxes)
    seg_os = tuple(-(-max(0, m - CAP) // 16) * 16 for m in maxes)
    pad_cmax = max(pads_c)
    OV = sum(seg_os)
    seg_off = [sum(seg_os[:s]) for s in range(BPC)]

    qe_m = query_embed * qmask[None, :, :, None].astype(np.float32)
    qidxs = [np.flatnonzero(qmask[g]) for g in range(BAT)]
    qpad = min(A, max(16, int(-(-max(len(q) for q in qidxs) // 16) * 16)))
    QW = BPC * L * qpad

    rts = np.ascontiguousarray(r.T * SCALE)          # [D, NB], fp32 bits

    idxs = [np.flatnonzero(dmask[g]) for g in range(BAT)]
    in_maps = []
    for c in range(CORES):
        # embeddings staged pre-transposed [D, tokens]; queries compacted
        # to their active rows (masks are per-batch, shared by both layers)
        qe_c = np.zeros((D, QW), dtype=np.float32)
        de_c = np.zeros((BPC, L, D, pad_cmax), dtype=np.float32)
        dov_c = np.zeros((D, 2 * OV), dtype=np.float32)
        for b in range(BPC):
            g = assign[c, b]
            qi = qidxs[g]
            for li in range(L):
                col = (b * L + li) * qpad
                qe_c[:, col:col + len(qi)] = qe_m[li, g, qi].T
            idx = idxs[g][:CAP]
            de_c[b, :, :, :len(idx)] = doc_embed[:, g, idx].transpose(0, 2, 1)
            ovi = idxs[g][CAP:]
            if len(ovi):
                for li in range(L):
                    c0 = li * OV + seg_off[b]
                    dov_c[:, c0:c0 + len(ovi)] = doc_embed[li, g, ovi].T
        m = {"qe": qe_c, "de": de_c, "rt": rts}
        if OV:
            m["dov"] = dov_c
        in_maps.append(m)

    return in_maps, assign, idxs, pads_c, seg_os, qidxs, qpad


def kernel(query_embed, doc_embed, query_tok, doc_tok, r):
    in_maps, assign, idxs, pads_c, seg_os, qidxs, qpad = _stage_inputs(
        query_embed, doc_embed, query_tok, doc_tok, r)
    OV = sum(seg_os)
    seg_off = [sum(seg_os[:s]) for s in range(BPC)]

    key = (pads_c, qpad, seg_os)
    if key not in _BUILD_CACHE:
        _BUILD_CACHE[key] = _build(pads_c, qpad, seg_os)
    nc = _BUILD_CACHE[key]

    res = run_bass_kernel_spmd(nc, in_maps, core_ids=list(range(CORES)))

    out = np.zeros((BAT, L, A, BDOC), dtype=np.float32)
    for c in range(CORES):
        o_c = res.results[c]["out"]  # [BPC, L, qpad, pad_cmax]
        o2_c = res.results[c].get("out2")
        for b in range(BPC):
            g = assign[c, b]
            idx = idxs[g][:CAP]
            qi = qidxs[g]
            for li in range(L):
                out[g, li][np.ix_(qi, idx)] = o_c[b, li, :len(qi), :len(idx)]
            ovi = idxs[g][CAP:]
            if len(ovi):
                for li in range(L):
                    c0 = li * OV + seg_off[b]
                    out[g, li][np.ix_(qi, ovi)] = \
                        o2_c[:len(qi), c0:c0 + len(ovi)]
    return out


# revision 26
# speedup vs baseline: 1.1373x; 1.0090x over previous
"""LSH similarity-matrix kernel for Trainium2 (8 NeuronCores, data-parallel over batch).

Math: reference computes, per (l, b):
    c1 = (query_embed @ r.T > 0),  c2 = (doc_embed @ r.T > 0)   in {0,1}
    ham = s1 + s2 - 2*c1@c2.T ;  sim = cos(pi/NB * ham), masked where tok==0.
With +-1 codes U = 2c-1 and S = U1 @ U2.T:  ham = (NB - S)/2, so
    sim = sin(pi/(2*NB) * S).
Masks fold into the embeddings: a zeroed embedding row projects to 0,
sign(0) = 0 gives a zero code row, so S = 0 and sin(0) = 0 — exactly the
masked output. Masked doc tokens (half of them: tok in {0,1}) are gathered
away host-side entirely; output columns scatter back as zeros.

Sharding: batches are assigned to (core, slot) sorted by active-token
count; every slot is capped at 512 docs so each chunk's projection fits a
single PSUM bank. Docs beyond 512 (a few tens per heavy batch) form one
extra "overflow" job per core: fixed per-slot segments, both layers side
by side, so the SPMD program shape is identical on every core.

Precision: the projection runs as a single float32r (TF32) matmul per
128-bit chunk. TF32's 11-bit mantissa flips ~1.4k of the 71M hash bits
(those whose fp32 projection sits within rounding error of zero), which
perturbs the final similarity by rel err ~7e-3 end-to-end — well inside
the 2e-2 gate — at 1/3 the PE cost of a compensated projection. The
embeddings and r stream from HBM straight into float32r tiles (f32r is
an engine compute mode over fp32 bits, so the DMA is a plain byte copy
and no on-device cast is needed). The code dot runs as fp8e4m3 DoubleRow
matmuls (chunk pairs give K=256 per MM at 0.5 cycles/row); +-1/0 codes
and their fp32 PSUM accumulation are exact.

The kernel is sign-throughput-bound: every projected bit crosses
PSUM->SBUF through DVE/ACT exactly once. So chunks are projected in
PAIRS into one 2-bank PSUM tile and signed by a single instruction
(halving per-instruction access overhead); pairs alternate between the
DVE (clamp) and ACT (Sign) engines, weighted so both engines carry equal
ns; query pairs interleave into the first two jobs' slack. r is
pre-scaled by 2^66 host-side so the DVE clamp(x,-1,1) = max(min(x,1),-1)
sign is exact (any |proj| > 2^-66 maps to +-1). DMAs ride independent
queues (loads: SP/HWDGE, stores: Pool SWDGE, which also bypasses the
shared HWDGE dispatcher for the first doc load) so a store waiting on
Sin never blocks the next job's load.
"""
import os
import sys

sys.path.insert(0, "/opt/trn_rl_repo")

from contextlib import ExitStack

import numpy as np

import concourse.bass as bass
import concourse.mybir as mybir
import concourse.tile as tile
from concourse import bacc
from concourse.bass_utils import run_bass_kernel_spmd

L, BAT, A, BDOC, D, NB = 2, 32, 64, 1024, 128, 1024
CORES = 8
BPC = BAT // CORES          # batch slots per core
CH = NB // 128              # 8 bit-chunks
NPAIR = CH // 2             # chunk pairs
CAP = 512                   # per-slot doc cap (one PSUM bank)
SCALE = float(2.0 ** 66)
PI = float(np.pi)

F32 = mybir.dt.float32
F32R = mybir.dt.float32r
BF16 = mybir.dt.bfloat16
FP8 = mybir.dt.float8e4
Alu = mybir.AluOpType
Act = mybir.ActivationFunctionType
DR = mybir.MatmulPerfMode.DoubleRow

_BUILD_CACHE: dict = {}

# chunk-pair sign engine: 1 = DVE clamp, 0 = ACT Sign (ACT also runs Sin)
_PAIR_DVE = (1, 0, 1, 0)
_QPAIR_DVE = (1, 0, 1, 1)   # 3 DVE / 1 ACT balances ACT's Sin load


def _build(pads_c: tuple, qpad: int, seg_os: tuple, reps: int = 1):
    """Per-core SPMD program. pads_c[b] <= CAP: width of main slot b;
    seg_os[b]: overflow segment width of slot b (0 = none). reps > 1
    re-emits the whole body (timing instrumentation only)."""
    pads_c = tuple(int(p) for p in pads_c)
    seg_os = tuple(int(s) for s in seg_os)
    pad_cmax = max(pads_c)
    OV = sum(seg_os)
    seg_off = [sum(seg_os[:s]) for s in range(BPC)]
    OV2 = 2 * OV                    # both layers side by side
    assert OV2 <= 512, "overflow exceeds one PSUM bank"

    nc = bacc.Bacc("TRN2", target_bir_lowering=False, debug=False)

    QW = BPC * L * qpad
    QE = nc.dram_tensor("qe", [D, QW], F32R, kind="ExternalInput").ap()
    DE = nc.dram_tensor("de", [BPC, L, D, pad_cmax], F32R,
                        kind="ExternalInput").ap()
    RT = nc.dram_tensor("rt", [D, NB], F32R, kind="ExternalInput").ap()
    OUT = nc.dram_tensor("out", [BPC, L, qpad, pad_cmax], F32,
                         kind="ExternalOutput").ap()
    if OV:
        DOV = nc.dram_tensor("dov", [D, OV2], F32R, kind="ExternalInput").ap()
        OUT2 = nc.dram_tensor("out2", [qpad, OV2], F32,
                              kind="ExternalOutput").ap()

    with tile.TileContext(nc) as tc, ExitStack() as ctx:
        const = ctx.enter_context(tc.tile_pool(name="const", bufs=1))
        jobp = ctx.enter_context(tc.tile_pool(name="jobp", bufs=4))
        outp = ctx.enter_context(tc.tile_pool(name="outp", bufs=2))
        # PSUM: chunk-pair tiles [128, 1024] (2 banks) x 3 bufs, plus the
        # dot-output tiles [*, 512] (1 bank) x 2 bufs = all 8 banks.
        ps_p = ctx.enter_context(tc.tile_pool(name="ps_p", bufs=3, space="PSUM"))

        for _rep in range(reps):
            _rp = f"r{_rep}_"
            # ---- constants, ordered for the serialized DMA-transfer queue:
            # SP/HWDGE carries the rt pieces (chunk 0-1 weights first so the
            # first projection unblocks earliest); the Pool SWDGE path
            # (bypasses the shared HWDGE dispatcher) carries the first doc
            # load + qe ----
            rt = const.tile([D, NB], F32R, tag="rt", name=f"{_rp}rt")
            nc.sync.dma_start(out=rt[:, 0:256], in_=RT[:, 0:256])
            nc.sync.dma_start(out=rt[:, 256:512], in_=RT[:, 256:512])
            qnat = const.tile([D, QW], F32R, tag="qnat", name=f"{_rp}qnat")

            # PE pre-warm: dependency-free dummy matmuls run while the first
            # DMAs land their completion receipts, pulling the PE through its
            # cold/mid clock ramp so the real projections start at 2.4 GHz
            warm = const.tile([D, 512], BF16, tag="warm", name=f"{_rp}warm")
            nc.gpsimd.memset(warm, 0.0)
            wps = ps_p.tile([D, 1024], F32, tag="pp",
                            name=f"{_rp}wps")[:, 0:512]
            for i in range(8):
                nc.tensor.matmul(wps, warm[:, 0:128], warm,
                                 start=True, stop=True)

            def load_consts_tail():
                nc.gpsimd.dma_start(out=qnat, in_=QE)
                nc.sync.dma_start(out=rt[:, 512:NB], in_=RT[:, 512:NB])

            U1 = const.tile([D, CH * QW], FP8, tag="U1", name=f"{_rp}U1")

            def _pair_sign(pp, dst2, wcols, dve):
                """One instruction signs both chunks of a pair: pp cols
                [0:w] and [512:512+w] -> dst2 [p, 2, w]."""
                if wcols == 512:
                    sv = pp[:]                                  # [p, 1024]
                    dv = dst2
                else:
                    sv = pp[:].rearrange("p (h c) -> p h c",
                                         h=2)[:, :, 0:wcols]
                    dv = dst2.rearrange("p (h c) -> p h c", h=2)
                if dve:
                    nc.vector.tensor_scalar(dv, sv, 1.0, -1.0,
                                            Alu.min, Alu.max)
                else:
                    nc.scalar.activation(dv, sv, Act.Sign)

            def query_grp(g):
                # query chunk pairs 2g, 2g+1; emitted inside the first two
                # jobs so the sign work fills both engines' slack
                for pr in (2 * g, 2 * g + 1):
                    qp = ps_p.tile([D, 1024], F32, tag="pp",
                                   name=f"{_rp}qp{pr}")
                    for h in (0, 1):
                        k = 2 * pr + h
                        nc.tensor.matmul(qp[:, h * 512:h * 512 + QW],
                                         rt[:, k * 128:(k + 1) * 128], qnat,
                                         start=True, stop=True)
                    _pair_sign(qp, U1[:, 2 * pr * QW:(2 * pr + 2) * QW],
                               QW, _QPAIR_DVE[pr])

            # ---- doc jobs, software-pipelined emission ----
            jobs = [(b, l) for b in range(BPC) for l in range(L)]
            n = len(jobs)
            st = [dict() for _ in range(n + 1)]     # [-1] = overflow job

            def stage_a(j):
                b, l = jobs[j]
                pad_c = pads_c[b]
                dnat = jobp.tile([D, pad_cmax], F32R, tag="dnat",
                                 name=f"{_rp}dnat{j}")[:, 0:pad_c]
                # job 0's load takes the Pool SWDGE path: it runs
                # concurrently with the rt pieces on SP
                eng = nc.gpsimd if j == 0 else nc.sync
                eng.dma_start(out=dnat, in_=DE[b, l, :, 0:pad_c])
                st[j]["e"] = dnat

            def stage_b(j):
                b, l = jobs[j]
                pad_c = pads_c[b]
                ev = st[j]["e"]
                U2 = jobp.tile([D, CH * pad_cmax], FP8, tag="U2",
                               name=f"{_rp}U2{j}")
                for pr in range(NPAIR):
                    pp = ps_p.tile([D, 1024], F32, tag="pp",
                                   name=f"{_rp}pp{j}_{pr}")
                    for h in (0, 1):
                        k = 2 * pr + h
                        nc.tensor.matmul(pp[:, h * 512:h * 512 + pad_c],
                                         rt[:, k * 128:(k + 1) * 128], ev,
                                         start=True, stop=True)
                    _pair_sign(pp, U2[:, 2 * pr * pad_c:(2 * pr + 2) * pad_c],
                               pad_c, _PAIR_DVE[pr])
                st[j]["U2"] = U2

            def _dot(S, U2, pad_c, qcol, c0, c1, p0):
                for jj in range(NPAIR):
                    lw = U1[:, 2 * jj * QW:(2 * jj + 2) * QW] \
                        .rearrange("p (o c) -> p o c", o=2)[:, :, qcol:qcol + qpad]
                    rv = U2[:, 2 * jj * pad_c:(2 * jj + 2) * pad_c] \
                        .rearrange("p (o c) -> p o c", o=2)[:, :, c0:c1]
                    nc.tensor.matmul(S[:, p0:p0 + c1 - c0], lw, rv,
                                     start=(jj == 0), stop=(jj == NPAIR - 1),
                                     perf_mode=DR)

            def stage_c(j, tail=False):
                b, l = jobs[j]
                pad_c = pads_c[b]
                U2 = st[j]["U2"]
                qcol = (b * L + l) * qpad
                sim = outp.tile([qpad, pad_cmax], F32, tag="sim",
                                name=f"{_rp}sim{j}")[:, 0:pad_c]
                if tail:
                    # last job (no overflow): per-half dot->sin->store with
                    # separate S tiles so the final store isn't serialized
                    # behind the full-width sin
                    hw2 = pad_c // 2
                    for pi, (c0, c1) in enumerate([(0, hw2), (hw2, pad_c)]):
                        S = ps_p.tile([qpad, 512], F32, tag="s",
                                      bufs=2, name=f"{_rp}St{pi}")
                        _dot(S, U2, pad_c, qcol, c0, c1, 0)
                        nc.scalar.activation(sim[:, c0:c1], S[:, 0:c1 - c0],
                                             Act.Sin, scale=PI / (2.0 * NB))
                        eng = nc.sync if pi else nc.gpsimd
                        eng.dma_start(out=OUT[b, l, :, c0:c1],
                                      in_=sim[:, c0:c1])
                    return
                S = ps_p.tile([qpad, 512], F32, tag="s", bufs=2,
                              name=f"{_rp}S{j}")
                _dot(S, U2, pad_c, qcol, 0, pad_c, 0)
                nc.scalar.activation(sim, S[:, 0:pad_c], Act.Sin,
                                     scale=PI / (2.0 * NB))
                # stores ride the Pool SWDGE queue: never block loads
                nc.gpsimd.dma_start(out=OUT[b, l, :, 0:pad_c], in_=sim)

            # ---- overflow job: one combined-layer job; segment s holds the
            # docs of slot s beyond CAP, layers side by side ----
            def stage_a_ov():
                dnat = jobp.tile([D, pad_cmax], F32R, tag="dnat",
                                 name=f"{_rp}dnatov")[:, 0:OV2]
                nc.sync.dma_start(out=dnat, in_=DOV)
                st[n]["e"] = dnat

            def stage_b_ov():
                ev = st[n]["e"]
                U2 = jobp.tile([D, CH * pad_cmax], FP8, tag="U2",
                               name=f"{_rp}U2ov")
                for pr in range(NPAIR):
                    pp = ps_p.tile([D, 1024], F32, tag="pp",
                                   name=f"{_rp}ppov{pr}")
                    for h in (0, 1):
                        k = 2 * pr + h
                        nc.tensor.matmul(pp[:, h * 512:h * 512 + OV2],
                                         rt[:, k * 128:(k + 1) * 128], ev,
                                         start=True, stop=True)
                    _pair_sign(pp, U2[:, 2 * pr * OV2:(2 * pr + 2) * OV2],
                               OV2, _PAIR_DVE[pr])
                st[n]["U2"] = U2

            def stage_c_ov():
                U2 = st[n]["U2"]
                S = ps_p.tile([qpad, 512], F32, tag="s", bufs=2,
                              name=f"{_rp}Sov")
                sim = outp.tile([qpad, pad_cmax], F32, tag="sim",
                                name=f"{_rp}simov")[:, 0:OV2]
                for li in range(L):
                    for s in range(BPC):
                        if not seg_os[s]:
                            continue
                        c0 = li * OV + seg_off[s]
                        qcol = (s * L + li) * qpad
                        _dot(S, U2, OV2, qcol, c0, c0 + seg_os[s], c0)
                nc.scalar.activation(sim, S[:, 0:OV2], Act.Sin,
                                     scale=PI / (2.0 * NB))
                nc.sync.dma_start(out=OUT2, in_=sim)

            stage_a(0)
            load_consts_tail()
            stage_b(0)
            query_grp(0)
            stage_a(1)
            query_grp(1)
            stage_a(2)
            stage_b(1)
            stage_a(3)
            for j in range(n):
                if OV and j == n - 3:
                    stage_b_ov()
                if j + 2 < n:
                    stage_b(j + 2)
                stage_c(j, tail=(not OV and j == n - 1))
                if j + 4 < n:
                    stage_a(j + 4)
                elif OV and j + 4 == n:
                    stage_a_ov()
            if OV:
                stage_c_ov()

    nc.compile()
    return nc


def _stage_inputs(query_embed, doc_embed, query_tok, doc_tok, r):
    query_embed = np.ascontiguousarray(query_embed, dtype=np.float32)
    doc_embed = np.ascontiguousarray(doc_embed, dtype=np.float32)
    r = np.ascontiguousarray(r, dtype=np.float32)

    qmask = (np.asarray(query_tok) != 0)
    dmask = (np.asarray(doc_tok) != 0)

    # sort batches by active count; slot s takes ranks [s*CORES, (s+1)*CORES)
    # spread across the 8 cores, so per-slot padding is tight and identical
    # on every core (SPMD requires one shape per slot)
    counts = dmask.sum(axis=1).astype(int)
    order = np.argsort(counts, kind="stable")
    assign = np.empty((CORES, BPC), dtype=int)   # assign[c, b] = batch id
    for s in range(BPC):
        for c in range(CORES):
            assign[c, s] = order[s * CORES + c]
    maxes = [int(counts[assign[:, s]].max()) for s in range(BPC)]
    pads_c = tuple(min(CAP, max(64, -(-m // 32) * 32)) for m in maxes)
    seg_os = tuple(-(-max(0, m - CAP) // 16) * 16 for m in maxes)
    pad_cmax = max(pads_c)
    OV = sum(seg_os)
    seg_off = [sum(seg_os[:s]) for s in range(BPC)]

    qe_m = query_embed * qmask[None, :, :, None].astype(np.float32)
    qidxs = [np.flatnonzero(qmask[g]) for g in range(BAT)]
    qpad = min(A, max(16, int(-(-max(len(q) for q in qidxs) // 16) * 16)))
    QW = BPC * L * qpad

    rts = np.ascontiguousarray(r.T * SCALE)          # [D, NB], fp32 bits

    idxs = [np.flatnonzero(dmask[g]) for g in range(BAT)]
    in_maps = []
    for c in range(CORES):
        # embeddings staged pre-transposed [D, tokens]; queries compacted
        # to their active rows (masks are per-batch, shared by both layers)
        qe_c = np.zeros((D, QW), dtype=np.float32)
        de_c = np.zeros((BPC, L, D, pad_cmax), dtype=np.float32)
        dov_c = np.zeros((D, 2 * OV), dtype=np.float32)
        for b in range(BPC):
            g = assign[c, b]
            qi = qidxs[g]
            for li in range(L):
                col = (b * L + li) * qpad
                qe_c[:, col:col + len(qi)] = qe_m[li, g, qi].T
            idx = idxs[g][:CAP]
            de_c[b, :, :, :len(idx)] = doc_embed[:, g, idx].transpose(0, 2, 1)
            ovi = idxs[g][CAP:]
            if len(ovi):
                for li in range(L):
                    c0 = li * OV + seg_off[b]
                    dov_c[:, c0:c0 + len(ovi)] = doc_embed[li, g, ovi].T
        m = {"qe": qe_c, "de": de_c, "rt": rts}
        if OV:
            m["dov"] = dov_c
        in_maps.append(m)

    return in_maps, assign, idxs, pads_c, seg_os, qidxs, qpad


def kernel(query_embed, doc_embed, query_tok, doc_tok, r):
    in_maps, assign, idxs, pads_c, seg_os, qidxs, qpad = _stage_inputs(
        query_embed, doc_embed, query_tok, doc_tok, r)
    OV = sum(seg_os)
    seg_off = [sum(seg_os[:s]) for s in range(BPC)]

    key = (pads_c, qpad, seg_os)
    if key not in _BUILD_CACHE:
        _BUILD_CACHE[key] = _build(pads_c, qpad, seg_os)
    nc = _BUILD_CACHE[key]

    res = run_bass_kernel_spmd(nc, in_maps, core_ids=list(range(CORES)))

    out = np.zeros((BAT, L, A, BDOC), dtype=np.float32)
    for c in range(CORES):
        o_c = res.results[c]["out"]  # [BPC, L, qpad, pad_cmax]
        o2_c = res.results[c].get("out2")
        for b in range(BPC):
            g = assign[c, b]
            idx = idxs[g][:CAP]
            qi = qidxs[g]
            for li in range(L):
                out[g, li][np.ix_(qi, idx)] = o_c[b, li, :len(qi), :len(idx)]
            ovi = idxs[g][CAP:]
            if len(ovi):
                for li in range(L):
                    c0 = li * OV + seg_off[b]
                    out[g, li][np.ix_(qi, ovi)] = \
                        o2_c[:len(qi), c0:c0 + len(ovi)]
    return out


# revision 28
# speedup vs baseline: 1.1559x; 1.0164x over previous
"""LSH similarity-matrix kernel for Trainium2 (8 NeuronCores, data-parallel over batch).

Math: reference computes, per (l, b):
    c1 = (query_embed @ r.T > 0),  c2 = (doc_embed @ r.T > 0)   in {0,1}
    ham = s1 + s2 - 2*c1@c2.T ;  sim = cos(pi/NB * ham), masked where tok==0.
With +-1 codes U = 2c-1 and S = U1 @ U2.T:  ham = (NB - S)/2, so
    sim = sin(pi/(2*NB) * S).
Masks fold into the embeddings: a zeroed embedding row projects to 0,
sign(0) = 0 gives a zero code row, so S = 0 and sin(0) = 0 — exactly the
masked output. Masked doc tokens (half of them: tok in {0,1}) are gathered
away host-side entirely; output columns scatter back as zeros.

Sharding: batches are assigned to (core, slot) sorted by active-token
count; every slot is capped at 512 docs so each chunk's projection fits a
single PSUM bank. Docs beyond 512 (a few tens per heavy batch) form one
extra "overflow" job per core: fixed per-slot segments, both layers side
by side, so the SPMD program shape is identical on every core.

Precision: the projection runs as a single float32r (TF32) matmul per
128-bit chunk. TF32's 11-bit mantissa flips ~1.4k of the 71M hash bits
(those whose fp32 projection sits within rounding error of zero), which
perturbs the final similarity by rel err ~7e-3 end-to-end — well inside
the 2e-2 gate — at 1/3 the PE cost of a compensated projection. The
embeddings and r stream from HBM straight into float32r tiles (f32r is
an engine compute mode over fp32 bits, so the DMA is a plain byte copy
and no on-device cast is needed). The code dot runs as fp8e4m3 DoubleRow
matmuls (chunk pairs give K=256 per MM at 0.5 cycles/row); +-1/0 codes
and their fp32 PSUM accumulation are exact.

The kernel is sign-throughput-bound: every projected bit crosses
PSUM->SBUF through DVE/ACT exactly once. So chunks are projected in
PAIRS into one 2-bank PSUM tile and signed by a single instruction
(halving per-instruction access overhead); pairs alternate between the
DVE (clamp) and ACT (Sign) engines, weighted so both engines carry equal
ns; query pairs interleave into the first two jobs' slack. r is
pre-scaled by 2^66 host-side so the DVE clamp(x,-1,1) = max(min(x,1),-1)
sign is exact (any |proj| > 2^-66 maps to +-1). DMAs ride independent
queues (loads: SP/HWDGE, stores: Pool SWDGE, which also bypasses the
shared HWDGE dispatcher for the first doc load) so a store waiting on
Sin never blocks the next job's load.
"""
import os
import sys

sys.path.insert(0, "/opt/trn_rl_repo")

from contextlib import ExitStack

import numpy as np

import concourse.bass as bass
import concourse.mybir as mybir
import concourse.tile as tile
from concourse import bacc
from concourse.bass_utils import run_bass_kernel_spmd

L, BAT, A, BDOC, D, NB = 2, 32, 64, 1024, 128, 1024
CORES = 8
BPC = BAT // CORES          # batch slots per core
CH = NB // 128              # 8 bit-chunks
NPAIR = CH // 2             # chunk pairs
CAP = 512                   # per-slot doc cap (one PSUM bank)
SCALE = float(2.0 ** 66)
PI = float(np.pi)

F32 = mybir.dt.float32
F32R = mybir.dt.float32r
BF16 = mybir.dt.bfloat16
FP8 = mybir.dt.float8e4
Alu = mybir.AluOpType
Act = mybir.ActivationFunctionType
DR = mybir.MatmulPerfMode.DoubleRow

_BUILD_CACHE: dict = {}

# chunk-pair sign engine: 1 = DVE clamp, 0 = ACT Sign (ACT also runs Sin)
_PAIR_DVE = (1, 0, 1, 0)
_QPAIR_DVE = (1, 0, 1, 1)   # 3 DVE / 1 ACT balances ACT's Sin load


def _build(pads_c: tuple, qpad: int, seg_os: tuple, reps: int = 1):
    """Per-core SPMD program. pads_c[b] <= CAP: width of main slot b;
    seg_os[b]: overflow segment width of slot b (0 = none). reps > 1
    re-emits the whole body (timing instrumentation only)."""
    pads_c = tuple(int(p) for p in pads_c)
    seg_os = tuple(int(s) for s in seg_os)
    pad_cmax = max(pads_c)
    OV = sum(seg_os)
    seg_off = [sum(seg_os[:s]) for s in range(BPC)]
    OV2 = 2 * OV                    # both layers side by side
    assert OV2 <= 512, "overflow exceeds one PSUM bank"

    nc = bacc.Bacc("TRN2", target_bir_lowering=False, debug=False)

    QW = BPC * L * qpad
    QE = nc.dram_tensor("qe", [D, QW], F32R, kind="ExternalInput").ap()
    DE = nc.dram_tensor("de", [BPC, L, D, pad_cmax], F32R,
                        kind="ExternalInput").ap()
    RT = nc.dram_tensor("rt", [D, NB], F32R, kind="ExternalInput").ap()
    OUT = nc.dram_tensor("out", [BPC, L, qpad, pad_cmax], F32,
                         kind="ExternalOutput").ap()
    if OV:
        DOV = nc.dram_tensor("dov", [D, OV2], F32R, kind="ExternalInput").ap()
        OUT2 = nc.dram_tensor("out2", [qpad, OV2], F32,
                              kind="ExternalOutput").ap()

    with tile.TileContext(nc) as tc, ExitStack() as ctx:
        const = ctx.enter_context(tc.tile_pool(name="const", bufs=1))
        jobp = ctx.enter_context(tc.tile_pool(name="jobp", bufs=4))
        outp = ctx.enter_context(tc.tile_pool(name="outp", bufs=4))
        # PSUM: chunk-pair tiles [128, 1024] (2 banks) x 3 bufs, plus the
        # dot-output tiles [*, 512] (1 bank) x 2 bufs = all 8 banks.
        ps_p = ctx.enter_context(tc.tile_pool(name="ps_p", bufs=3, space="PSUM"))

        for _rep in range(reps):
            _rp = f"r{_rep}_"
            # ---- constants, ordered for the serialized DMA-transfer queue:
            # SP/HWDGE carries the rt pieces (chunk 0-1 weights first so the
            # first projection unblocks earliest); the Pool SWDGE path
            # (bypasses the shared HWDGE dispatcher) carries the first doc
            # load + qe ----
            rt = const.tile([D, NB], F32R, tag="rt", name=f"{_rp}rt")
            nc.sync.dma_start(out=rt[:, 0:256], in_=RT[:, 0:256])
            nc.sync.dma_start(out=rt[:, 256:512], in_=RT[:, 256:512])
            qnat = const.tile([D, QW], F32R, tag="qnat", name=f"{_rp}qnat")

            # PE pre-warm: dependency-free dummy matmuls run while the first
            # DMAs land their completion receipts, pulling the PE through its
            # cold/mid clock ramp so the real projections start at 2.4 GHz
            warm = const.tile([D, 512], BF16, tag="warm", name=f"{_rp}warm")
            nc.gpsimd.memset(warm, 0.0)
            wps = ps_p.tile([D, 1024], F32, tag="pp",
                            name=f"{_rp}wps")[:, 0:512]
            for i in range(8):
                nc.tensor.matmul(wps, warm[:, 0:128], warm,
                                 start=True, stop=True)

            def load_consts_tail():
                nc.gpsimd.dma_start(out=qnat, in_=QE)
                nc.sync.dma_start(out=rt[:, 512:NB], in_=RT[:, 512:NB])

            U1 = const.tile([D, CH * QW], FP8, tag="U1", name=f"{_rp}U1")

            def _pair_sign(pp, dst2, wcols, dve):
                """One instruction signs both chunks of a pair: pp cols
                [0:w] and [512:512+w] -> dst2 [p, 2, w]."""
                if wcols == 512:
                    sv = pp[:]                                  # [p, 1024]
                    dv = dst2
                else:
                    sv = pp[:].rearrange("p (h c) -> p h c",
                                         h=2)[:, :, 0:wcols]
                    dv = dst2.rearrange("p (h c) -> p h c", h=2)
                if dve:
                    nc.vector.tensor_scalar(dv, sv, 1.0, -1.0,
                                            Alu.min, Alu.max)
                else:
                    nc.scalar.activation(dv, sv, Act.Sign)

            def query_grp(g):
                # query chunk pairs 2g, 2g+1; emitted inside the first two
                # jobs so the sign work fills both engines' slack
                for pr in (2 * g, 2 * g + 1):
                    qp = ps_p.tile([D, 1024], F32, tag="pp",
                                   name=f"{_rp}qp{pr}")
                    for h in (0, 1):
                        k = 2 * pr + h
                        nc.tensor.matmul(qp[:, h * 512:h * 512 + QW],
                                         rt[:, k * 128:(k + 1) * 128], qnat,
                                         start=True, stop=True)
                    _pair_sign(qp, U1[:, 2 * pr * QW:(2 * pr + 2) * QW],
                               QW, _QPAIR_DVE[pr])

            # ---- doc jobs, software-pipelined emission ----
            jobs = [(b, l) for b in range(BPC) for l in range(L)]
            n = len(jobs)
            st = [dict() for _ in range(n + 1)]     # [-1] = overflow job

            def stage_a(j):
                b, l = jobs[j]
                pad_c = pads_c[b]
                dnat = jobp.tile([D, pad_cmax], F32R, tag="dnat",
                                 name=f"{_rp}dnat{j}")[:, 0:pad_c]
                # job 0's load takes the Pool SWDGE path: it runs
                # concurrently with the rt pieces on SP
                eng = nc.gpsimd if j == 0 else nc.sync
                eng.dma_start(out=dnat, in_=DE[b, l, :, 0:pad_c])
                st[j]["e"] = dnat

            def stage_b(j):
                b, l = jobs[j]
                pad_c = pads_c[b]
                ev = st[j]["e"]
                U2 = jobp.tile([D, CH * pad_cmax], FP8, tag="U2",
                               name=f"{_rp}U2{j}")
                for pr in range(NPAIR):
                    pp = ps_p.tile([D, 1024], F32, tag="pp",
                                   name=f"{_rp}pp{j}_{pr}")
                    for h in (0, 1):
                        k = 2 * pr + h
                        nc.tensor.matmul(pp[:, h * 512:h * 512 + pad_c],
                                         rt[:, k * 128:(k + 1) * 128], ev,
                                         start=True, stop=True)
                    _pair_sign(pp, U2[:, 2 * pr * pad_c:(2 * pr + 2) * pad_c],
                               pad_c, _PAIR_DVE[pr])
                st[j]["U2"] = U2

            def _dot(S, U2, pad_c, qcol, c0, c1, p0):
                for jj in range(NPAIR):
                    lw = U1[:, 2 * jj * QW:(2 * jj + 2) * QW] \
                        .rearrange("p (o c) -> p o c", o=2)[:, :, qcol:qcol + qpad]
                    rv = U2[:, 2 * jj * pad_c:(2 * jj + 2) * pad_c] \
                        .rearrange("p (o c) -> p o c", o=2)[:, :, c0:c1]
                    nc.tensor.matmul(S[:, p0:p0 + c1 - c0], lw, rv,
                                     start=(jj == 0), stop=(jj == NPAIR - 1),
                                     perf_mode=DR)

            def stage_c(j, tail=False):
                b, l = jobs[j]
                pad_c = pads_c[b]
                U2 = st[j]["U2"]
                qcol = (b * L + l) * qpad
                sim = outp.tile([qpad, pad_cmax], F32, tag="sim",
                                name=f"{_rp}sim{j}")[:, 0:pad_c]
                if tail:
                    # last job (no overflow): per-half dot->sin->store with
                    # separate S tiles so the final store isn't serialized
                    # behind the full-width sin
                    hw2 = pad_c // 2
                    for pi, (c0, c1) in enumerate([(0, hw2), (hw2, pad_c)]):
                        S = ps_p.tile([qpad, 512], F32, tag="s",
                                      bufs=2, name=f"{_rp}St{pi}")
                        _dot(S, U2, pad_c, qcol, c0, c1, 0)
                        nc.scalar.activation(sim[:, c0:c1], S[:, 0:c1 - c0],
                                             Act.Sin, scale=PI / (2.0 * NB))
                        eng = nc.sync if pi else nc.gpsimd
                        eng.dma_start(out=OUT[b, l, :, c0:c1],
                                      in_=sim[:, c0:c1])
                    return
                S = ps_p.tile([qpad, 512], F32, tag="s", bufs=2,
                              name=f"{_rp}S{j}")
                _dot(S, U2, pad_c, qcol, 0, pad_c, 0)
                nc.scalar.activation(sim, S[:, 0:pad_c], Act.Sin,
                                     scale=PI / (2.0 * NB))
                # stores ride the Pool SWDGE queue (never block loads); the
                # final job's store takes SP's lower-latency hwdge path
                eng = nc.sync if j == n - 1 else nc.gpsimd
                eng.dma_start(out=OUT[b, l, :, 0:pad_c], in_=sim)

            # ---- overflow job: one combined-layer job; segment s holds the
            # docs of slot s beyond CAP, layers side by side ----
            def stage_a_ov():
                dnat = jobp.tile([D, pad_cmax], F32R, tag="dnat",
                                 name=f"{_rp}dnatov")[:, 0:OV2]
                nc.sync.dma_start(out=dnat, in_=DOV)
                st[n]["e"] = dnat

            def stage_b_ov():
                ev = st[n]["e"]
                U2 = jobp.tile([D, CH * pad_cmax], FP8, tag="U2",
                               name=f"{_rp}U2ov")
                for pr in range(NPAIR):
                    pp = ps_p.tile([D, 1024], F32, tag="pp",
                                   name=f"{_rp}ppov{pr}")
                    for h in (0, 1):
                        k = 2 * pr + h
                        nc.tensor.matmul(pp[:, h * 512:h * 512 + OV2],
                                         rt[:, k * 128:(k + 1) * 128], ev,
                                         start=True, stop=True)
                    _pair_sign(pp, U2[:, 2 * pr * OV2:(2 * pr + 2) * OV2],
                               OV2, _PAIR_DVE[pr])
                st[n]["U2"] = U2

            def stage_c_ov():
                U2 = st[n]["U2"]
                S = ps_p.tile([qpad, 512], F32, tag="s", bufs=2,
                              name=f"{_rp}Sov")
                sim = outp.tile([qpad, pad_cmax], F32, tag="sim",
                                name=f"{_rp}simov")[:, 0:OV2]
                for li in range(L):
                    for s in range(BPC):
                        if not seg_os[s]:
                            continue
                        c0 = li * OV + seg_off[s]
                        qcol = (s * L + li) * qpad
                        _dot(S, U2, OV2, qcol, c0, c0 + seg_os[s], c0)
                nc.scalar.activation(sim, S[:, 0:OV2], Act.Sin,
                                     scale=PI / (2.0 * NB))
                nc.sync.dma_start(out=OUT2, in_=sim)

            stage_a(0)
            load_consts_tail()
            stage_b(0)
            query_grp(0)
            stage_a(1)
            query_grp(1)
            stage_a(2)
            stage_b(1)
            stage_a(3)
            for j in range(n):
                if OV and j == n - 3:
                    stage_b_ov()
                if j + 2 < n:
                    stage_b(j + 2)
                stage_c(j, tail=(not OV and j == n - 1))
                if j + 4 < n:
                    stage_a(j + 4)
                elif OV and j + 4 == n:
                    stage_a_ov()
            if OV:
                stage_c_ov()

    nc.compile()
    return nc


def _stage_inputs(query_embed, doc_embed, query_tok, doc_tok, r):
    query_embed = np.ascontiguousarray(query_embed, dtype=np.float32)
    doc_embed = np.ascontiguousarray(doc_embed, dtype=np.float32)
    r = np.ascontiguousarray(r, dtype=np.float32)

    qmask = (np.asarray(query_tok) != 0)
    dmask = (np.asarray(doc_tok) != 0)

    # sort batches by active count; slot s takes ranks [s*CORES, (s+1)*CORES)
    # spread across the 8 cores, so per-slot padding is tight and identical
    # on every core (SPMD requires one shape per slot)
    counts = dmask.sum(axis=1).astype(int)
    order = np.argsort(counts, kind="stable")
    assign = np.empty((CORES, BPC), dtype=int)   # assign[c, b] = batch id
    for s in range(BPC):
        for c in range(CORES):
            assign[c, s] = order[s * CORES + c]
    maxes = [int(counts[assign[:, s]].max()) for s in range(BPC)]
    pads_c = tuple(min(CAP, max(64, -(-m // 32) * 32)) for m in maxes)
    seg_os = tuple(-(-max(0, m - CAP) // 16) * 16 for m in maxes)
    pad_cmax = max(pads_c)
    OV = sum(seg_os)
    seg_off = [sum(seg_os[:s]) for s in range(BPC)]

    qe_m = query_embed * qmask[None, :, :, None].astype(np.float32)
    qidxs = [np.flatnonzero(qmask[g]) for g in range(BAT)]
    qpad = min(A, max(16, int(-(-max(len(q) for q in qidxs) // 16) * 16)))
    QW = BPC * L * qpad

    rts = np.ascontiguousarray(r.T * SCALE)          # [D, NB], fp32 bits

    idxs = [np.flatnonzero(dmask[g]) for g in range(BAT)]
    in_maps = []
    for c in range(CORES):
        # embeddings staged pre-transposed [D, tokens]; queries compacted
        # to their active rows (masks are per-batch, shared by both layers)
        qe_c = np.zeros((D, QW), dtype=np.float32)
        de_c = np.zeros((BPC, L, D, pad_cmax), dtype=np.float32)
        dov_c = np.zeros((D, 2 * OV), dtype=np.float32)
        for b in range(BPC):
            g = assign[c, b]
            qi = qidxs[g]
            for li in range(L):
                col = (b * L + li) * qpad
                qe_c[:, col:col + len(qi)] = qe_m[li, g, qi].T
            idx = idxs[g][:CAP]
            de_c[b, :, :, :len(idx)] = doc_embed[:, g, idx].transpose(0, 2, 1)
            ovi = idxs[g][CAP:]
            if len(ovi):
                for li in range(L):
                    c0 = li * OV + seg_off[b]
                    dov_c[:, c0:c0 + len(ovi)] = doc_embed[li, g, ovi].T
        m = {"qe": qe_c, "de": de_c, "rt": rts}
        if OV:
            m["dov"] = dov_c
        in_maps.append(m)

    return in_maps, assign, idxs, pads_c, seg_os, qidxs, qpad


def kernel(query_embed, doc_embed, query_tok, doc_tok, r):
    in_maps, assign, idxs, pads_c, seg_os, qidxs, qpad = _stage_inputs(
        query_embed, doc_embed, query_tok, doc_tok, r)
    OV = sum(seg_os)
    seg_off = [sum(seg_os[:s]) for s in range(BPC)]

    key = (pads_c, qpad, seg_os)
    if key not in _BUILD_CACHE:
        _BUILD_CACHE[key] = _build(pads_c, qpad, seg_os)
    nc = _BUILD_CACHE[key]

    res = run_bass_kernel_spmd(nc, in_maps, core_ids=list(range(CORES)))

    out = np.zeros((BAT, L, A, BDOC), dtype=np.float32)
    for c in range(CORES):
        o_c = res.results[c]["out"]  # [BPC, L, qpad, pad_cmax]
        o2_c = res.results[c].get("out2")
        for b in range(BPC):
            g = assign[c, b]
            idx = idxs[g][:CAP]
            qi = qidxs[g]
            for li in range(L):
                out[g, li][np.ix_(qi, idx)] = o_c[b, li, :len(qi), :len(idx)]
            ovi = idxs[g][CAP:]
            if len(ovi):
                for li in range(L):
                    c0 = li * OV + seg_off[b]
                    out[g, li][np.ix_(qi, ovi)] = \
                        o2_c[:len(qi), c0:c0 + len(ovi)]
    return out


# revision 32
# speedup vs baseline: 1.1653x; 1.0081x over previous
"""LSH similarity-matrix kernel for Trainium2 (8 NeuronCores, data-parallel over batch).

Math: reference computes, per (l, b):
    c1 = (query_embed @ r.T > 0),  c2 = (doc_embed @ r.T > 0)   in {0,1}
    ham = s1 + s2 - 2*c1@c2.T ;  sim = cos(pi/NB * ham), masked where tok==0.
With +-1 codes U = 2c-1 and S = U1 @ U2.T:  ham = (NB - S)/2, so
    sim = sin(pi/(2*NB) * S).
Masks fold into the embeddings: a zeroed embedding row projects to 0,
sign(0) = 0 gives a zero code row, so S = 0 and sin(0) = 0 — exactly the
masked output. Masked doc tokens (half of them: tok in {0,1}) are gathered
away host-side entirely; output columns scatter back as zeros.

Sharding: batches are assigned to (core, slot) sorted by active-token
count; every slot is capped at 512 docs so each chunk's projection fits a
single PSUM bank. Docs beyond 512 (a few tens per heavy batch) form one
extra "overflow" job per core: fixed per-slot segments, both layers side
by side, so the SPMD program shape is identical on every core.

Precision: the projection runs as a single float32r (TF32) matmul per
128-bit chunk. TF32's 11-bit mantissa flips ~1.4k of the 71M hash bits
(those whose fp32 projection sits within rounding error of zero), which
perturbs the final similarity by rel err ~7e-3 end-to-end — well inside
the 2e-2 gate — at 1/3 the PE cost of a compensated projection. The
embeddings and r stream from HBM straight into float32r tiles (f32r is
an engine compute mode over fp32 bits, so the DMA is a plain byte copy
and no on-device cast is needed). The code dot runs as fp8e4m3 DoubleRow
matmuls (chunk pairs give K=256 per MM at 0.5 cycles/row); +-1/0 codes
and their fp32 PSUM accumulation are exact.

The kernel is sign-throughput-bound: every projected bit crosses
PSUM->SBUF through DVE/ACT exactly once. So chunks are projected in
PAIRS into one 2-bank PSUM tile and signed by a single instruction
(halving per-instruction access overhead); pairs alternate between the
DVE (clamp) and ACT (Sign) engines, weighted so both engines carry equal
ns; query pairs interleave into the first two jobs' slack. r is
pre-scaled by 2^66 host-side so the DVE clamp(x,-1,1) = max(min(x,1),-1)
sign is exact (any |proj| > 2^-66 maps to +-1). DMAs ride independent
queues (loads: SP/HWDGE, stores: Pool SWDGE, which also bypasses the
shared HWDGE dispatcher for the first doc load) so a store waiting on
Sin never blocks the next job's load.
"""
import os
import sys

sys.path.insert(0, "/opt/trn_rl_repo")

from contextlib import ExitStack

import numpy as np

import concourse.bass as bass
import concourse.mybir as mybir
import concourse.tile as tile
from concourse import bacc
from concourse.bass_utils import run_bass_kernel_spmd

L, BAT, A, BDOC, D, NB = 2, 32, 64, 1024, 128, 1024
CORES = 8
BPC = BAT // CORES          # batch slots per core
CH = NB // 128              # 8 bit-chunks
NPAIR = CH // 2             # chunk pairs
CAP = 512                   # per-slot doc cap (one PSUM bank)
SCALE = float(2.0 ** 66)
PI = float(np.pi)

F32 = mybir.dt.float32
F32R = mybir.dt.float32r
BF16 = mybir.dt.bfloat16
FP8 = mybir.dt.float8e4
Alu = mybir.AluOpType
Act = mybir.ActivationFunctionType
DR = mybir.MatmulPerfMode.DoubleRow

_BUILD_CACHE: dict = {}

# chunk-pair sign engine: 1 = DVE clamp, 0 = ACT Sign (ACT also runs Sin)
_PAIR_DVE = (1, 0, 1, 0)
_QPAIR_DVE = (1, 0, 1, 1)   # 3 DVE / 1 ACT balances ACT's Sin load


def _build(pads_c: tuple, qpad: int, seg_os: tuple, reps: int = 1):
    """Per-core SPMD program. pads_c[b] <= CAP: width of main slot b;
    seg_os[b]: overflow segment width of slot b (0 = none). reps > 1
    re-emits the whole body (timing instrumentation only)."""
    pads_c = tuple(int(p) for p in pads_c)
    seg_os = tuple(int(s) for s in seg_os)
    pad_cmax = max(pads_c)
    OV = sum(seg_os)
    seg_off = [sum(seg_os[:s]) for s in range(BPC)]
    OV2 = 2 * OV                    # both layers side by side
    assert OV2 <= 512, "overflow exceeds one PSUM bank"

    nc = bacc.Bacc("TRN2", target_bir_lowering=False, debug=False)

    QW = BPC * L * qpad
    QE = nc.dram_tensor("qe", [D, QW], F32R, kind="ExternalInput").ap()
    DE = nc.dram_tensor("de", [BPC, L, D, pad_cmax], F32R,
                        kind="ExternalInput").ap()
    RT = nc.dram_tensor("rt", [D, NB], F32R, kind="ExternalInput").ap()
    OUT = nc.dram_tensor("out", [BPC, L, qpad, pad_cmax], F32,
                         kind="ExternalOutput").ap()
    if OV:
        DOV = nc.dram_tensor("dov", [D, OV2], F32R, kind="ExternalInput").ap()
        OUT2 = nc.dram_tensor("out2", [qpad, OV2], F32,
                              kind="ExternalOutput").ap()

    with tile.TileContext(nc) as tc, ExitStack() as ctx:
        const = ctx.enter_context(tc.tile_pool(name="const", bufs=1))
        jobp = ctx.enter_context(tc.tile_pool(name="jobp", bufs=4))
        outp = ctx.enter_context(tc.tile_pool(name="outp", bufs=4))
        # PSUM: chunk-pair tiles [128, 1024] (2 banks) x 3 bufs, plus the
        # dot-output tiles [*, 512] (1 bank) x 2 bufs = all 8 banks.
        ps_p = ctx.enter_context(tc.tile_pool(name="ps_p", bufs=3, space="PSUM"))

        for _rep in range(reps):
            _rp = f"r{_rep}_"
            # ---- constants, ordered for the serialized DMA-transfer queue:
            # SP/HWDGE carries the rt pieces (chunk 0-1 weights first so the
            # first projection unblocks earliest); the Pool SWDGE path
            # (bypasses the shared HWDGE dispatcher) carries the first doc
            # load + qe ----
            rt = const.tile([D, NB], F32R, tag="rt", name=f"{_rp}rt")
            nc.sync.dma_start(out=rt[:, 0:256], in_=RT[:, 0:256])
            nc.sync.dma_start(out=rt[:, 256:512], in_=RT[:, 256:512])
            qnat = const.tile([D, QW], F32R, tag="qnat", name=f"{_rp}qnat")

            # PE pre-warm: dependency-free dummy matmuls run while the first
            # DMAs land their completion receipts, pulling the PE through its
            # cold/mid clock ramp so the real projections start at 2.4 GHz.
            # warm's memset rides the (idle until ~4.5us) DVE so the Pool
            # engine can start generating the first doc load immediately.
            warm = const.tile([D, 512], BF16, tag="warm", name=f"{_rp}warm")
            nc.vector.memset(warm, 0.0)
            wps = ps_p.tile([D, 1024], F32, tag="pp",
                            name=f"{_rp}wps")[:, 0:512]
            for i in range(8):
                nc.tensor.matmul(wps, warm[:, 0:128], warm,
                                 start=True, stop=True)

            def load_consts_tail():
                nc.sync.dma_start(out=qnat, in_=QE)
                nc.sync.dma_start(out=rt[:, 512:NB], in_=RT[:, 512:NB])

            U1 = const.tile([D, CH * QW], FP8, tag="U1", name=f"{_rp}U1")

            def _pair_sign(pp, dst2, wcols, dve):
                """One instruction signs both chunks of a pair: pp cols
                [0:w] and [512:512+w] -> dst2 [p, 2, w]."""
                if wcols == 512:
                    sv = pp[:]                                  # [p, 1024]
                    dv = dst2
                else:
                    sv = pp[:].rearrange("p (h c) -> p h c",
                                         h=2)[:, :, 0:wcols]
                    dv = dst2.rearrange("p (h c) -> p h c", h=2)
                if dve:
                    nc.vector.tensor_scalar(dv, sv, 1.0, -1.0,
                                            Alu.min, Alu.max)
                else:
                    nc.scalar.activation(dv, sv, Act.Sign)

            def query_grp(g):
                # query chunk pairs 2g, 2g+1; emitted inside the first two
                # jobs so the sign work fills both engines' slack
                for pr in (2 * g, 2 * g + 1):
                    qp = ps_p.tile([D, 1024], F32, tag="pp",
                                   name=f"{_rp}qp{pr}")
                    for h in (0, 1):
                        k = 2 * pr + h
                        nc.tensor.matmul(qp[:, h * 512:h * 512 + QW],
                                         rt[:, k * 128:(k + 1) * 128], qnat,
                                         start=True, stop=True)
                    _pair_sign(qp, U1[:, 2 * pr * QW:(2 * pr + 2) * QW],
                               QW, _QPAIR_DVE[pr])

            # ---- doc jobs, software-pipelined emission ----
            jobs = [(b, l) for b in range(BPC) for l in range(L)]
            n = len(jobs)
            st = [dict() for _ in range(n + 1)]     # [-1] = overflow job

            def stage_a(j):
                b, l = jobs[j]
                pad_c = pads_c[b]
                dnat = jobp.tile([D, pad_cmax], F32R, tag="dnat",
                                 name=f"{_rp}dnat{j}")[:, 0:pad_c]
                # job 0's load takes the Pool SWDGE path: it runs
                # concurrently with the rt pieces on SP
                eng = nc.gpsimd if j == 0 else nc.sync
                eng.dma_start(out=dnat, in_=DE[b, l, :, 0:pad_c])
                st[j]["e"] = dnat

            def stage_b(j, prs=None):
                b, l = jobs[j]
                pad_c = pads_c[b]
                ev = st[j]["e"]
                if prs is None or prs[0] == 0:
                    st[j]["U2"] = jobp.tile([D, CH * pad_cmax], FP8, tag="U2",
                                            name=f"{_rp}U2{j}")
                U2 = st[j]["U2"]
                for pr in (range(NPAIR) if prs is None else prs):
                    pp = ps_p.tile([D, 1024], F32, tag="pp",
                                   name=f"{_rp}pp{j}_{pr}")
                    for h in (0, 1):
                        k = 2 * pr + h
                        nc.tensor.matmul(pp[:, h * 512:h * 512 + pad_c],
                                         rt[:, k * 128:(k + 1) * 128], ev,
                                         start=True, stop=True)
                    _pair_sign(pp, U2[:, 2 * pr * pad_c:(2 * pr + 2) * pad_c],
                               pad_c, _PAIR_DVE[pr])

            def _dot(S, U2, pad_c, qcol, c0, c1, p0):
                for jj in range(NPAIR):
                    lw = U1[:, 2 * jj * QW:(2 * jj + 2) * QW] \
                        .rearrange("p (o c) -> p o c", o=2)[:, :, qcol:qcol + qpad]
                    rv = U2[:, 2 * jj * pad_c:(2 * jj + 2) * pad_c] \
                        .rearrange("p (o c) -> p o c", o=2)[:, :, c0:c1]
                    nc.tensor.matmul(S[:, p0:p0 + c1 - c0], lw, rv,
                                     start=(jj == 0), stop=(jj == NPAIR - 1),
                                     perf_mode=DR)

            def stage_c(j, tail=False):
                b, l = jobs[j]
                pad_c = pads_c[b]
                U2 = st[j]["U2"]
                qcol = (b * L + l) * qpad
                sim = outp.tile([qpad, pad_cmax], F32, tag="sim",
                                name=f"{_rp}sim{j}")[:, 0:pad_c]
                if tail:
                    # last job (no overflow): per-half dot->sin->store with
                    # separate S tiles so the final store isn't serialized
                    # behind the full-width sin
                    hw2 = pad_c // 2
                    for pi, (c0, c1) in enumerate([(0, hw2), (hw2, pad_c)]):
                        S = ps_p.tile([qpad, 512], F32, tag="s",
                                      bufs=2, name=f"{_rp}St{pi}")
                        _dot(S, U2, pad_c, qcol, c0, c1, 0)
                        nc.scalar.activation(sim[:, c0:c1], S[:, 0:c1 - c0],
                                             Act.Sin, scale=PI / (2.0 * NB))
                        eng = nc.sync if pi else nc.gpsimd
                        eng.dma_start(out=OUT[b, l, :, c0:c1],
                                      in_=sim[:, c0:c1])
                    return
                S = ps_p.tile([qpad, 512], F32, tag="s", bufs=2,
                              name=f"{_rp}S{j}")
                _dot(S, U2, pad_c, qcol, 0, pad_c, 0)
                nc.scalar.activation(sim, S[:, 0:pad_c], Act.Sin,
                                     scale=PI / (2.0 * NB))
                # stores ride the Pool SWDGE queue (never block loads); the
                # final job's store takes SP's lower-latency hwdge path
                eng = nc.sync if j == n - 1 else nc.gpsimd
                eng.dma_start(out=OUT[b, l, :, 0:pad_c], in_=sim)

            # ---- overflow job: one combined-layer job; segment s holds the
            # docs of slot s beyond CAP, layers side by side ----
            def stage_a_ov():
                dnat = jobp.tile([D, pad_cmax], F32R, tag="dnat",
                                 name=f"{_rp}dnatov")[:, 0:OV2]
                nc.sync.dma_start(out=dnat, in_=DOV)
                st[n]["e"] = dnat

            def stage_b_ov():
                ev = st[n]["e"]
                U2 = jobp.tile([D, CH * pad_cmax], FP8, tag="U2",
                               name=f"{_rp}U2ov")
                for pr in range(NPAIR):
                    pp = ps_p.tile([D, 1024], F32, tag="pp",
                                   name=f"{_rp}ppov{pr}")
                    for h in (0, 1):
                        k = 2 * pr + h
                        nc.tensor.matmul(pp[:, h * 512:h * 512 + OV2],
                                         rt[:, k * 128:(k + 1) * 128], ev,
                                         start=True, stop=True)
                    _pair_sign(pp, U2[:, 2 * pr * OV2:(2 * pr + 2) * OV2],
                               OV2, _PAIR_DVE[pr])
                st[n]["U2"] = U2

            def stage_c_ov():
                U2 = st[n]["U2"]
                S = ps_p.tile([qpad, 512], F32, tag="s", bufs=2,
                              name=f"{_rp}Sov")
                sim = outp.tile([qpad, pad_cmax], F32, tag="sim",
                                name=f"{_rp}simov")[:, 0:OV2]
                for li in range(L):
                    for s in range(BPC):
                        if not seg_os[s]:
                            continue
                        c0 = li * OV + seg_off[s]
                        qcol = (s * L + li) * qpad
                        _dot(S, U2, OV2, qcol, c0, c0 + seg_os[s], c0)
                nc.scalar.activation(sim, S[:, 0:OV2], Act.Sin,
                                     scale=PI / (2.0 * NB))
                nc.sync.dma_start(out=OUT2, in_=sim)

            stage_a(0)
            load_consts_tail()
            # job 0's pairs interleave with the query pairs so both sign
            # engines run continuously from the first DMA landing: pairs
            # 0-1 + query chunks 0-3 need only rt[:512]+qe; the rest rt[512:]
            stage_b(0, prs=(0, 1))
            query_grp(0)
            stage_b(0, prs=(2, 3))
            query_grp(1)
            stage_a(1)
            stage_a(2)
            stage_b(1)
            stage_a(3)
            for j in range(n):
                if OV and j == n - 3:
                    stage_b_ov()
                if j + 2 < n:
                    stage_b(j + 2)
                stage_c(j, tail=(not OV and j == n - 1))
                if j + 4 < n:
                    stage_a(j + 4)
                elif OV and j + 4 == n:
                    stage_a_ov()
            if OV:
                stage_c_ov()

    nc.compile()
    return nc


def _stage_inputs(query_embed, doc_embed, query_tok, doc_tok, r):
    query_embed = np.ascontiguousarray(query_embed, dtype=np.float32)
    doc_embed = np.ascontiguousarray(doc_embed, dtype=np.float32)
    r = np.ascontiguousarray(r, dtype=np.float32)

    qmask = (np.asarray(query_tok) != 0)
    dmask = (np.asarray(doc_tok) != 0)

    # sort batches by active count; slot s takes ranks [s*CORES, (s+1)*CORES)
    # spread across the 8 cores, so per-slot padding is tight and identical
    # on every core (SPMD requires one shape per slot)
    counts = dmask.sum(axis=1).astype(int)
    order = np.argsort(counts, kind="stable")
    assign = np.empty((CORES, BPC), dtype=int)   # assign[c, b] = batch id
    for s in range(BPC):
        for c in range(CORES):
            assign[c, s] = order[s * CORES + c]
    maxes = [int(counts[assign[:, s]].max()) for s in range(BPC)]
    pads_c = tuple(min(CAP, max(64, -(-m // 32) * 32)) for m in maxes)
    seg_os = tuple(-(-max(0, m - CAP) // 16) * 16 for m in maxes)
    pad_cmax = max(pads_c)
    OV = sum(seg_os)
    seg_off = [sum(seg_os[:s]) for s in range(BPC)]

    qe_m = query_embed * qmask[None, :, :, None].astype(np.float32)
    qidxs = [np.flatnonzero(qmask[g]) for g in range(BAT)]
    qpad = min(A, max(16, int(-(-max(len(q) for q in qidxs) // 16) * 16)))
    QW = BPC * L * qpad

    rts = np.ascontiguousarray(r.T * SCALE)          # [D, NB], fp32 bits

    idxs = [np.flatnonzero(dmask[g]) for g in range(BAT)]
    in_maps = []
    for c in range(CORES):
        # embeddings staged pre-transposed [D, tokens]; queries compacted
        # to their active rows (masks are per-batch, shared by both layers)
        qe_c = np.zeros((D, QW), dtype=np.float32)
        de_c = np.zeros((BPC, L, D, pad_cmax), dtype=np.float32)
        dov_c = np.zeros((D, 2 * OV), dtype=np.float32)
        for b in range(BPC):
            g = assign[c, b]
            qi = qidxs[g]
            for li in range(L):
                col = (b * L + li) * qpad
                qe_c[:, col:col + len(qi)] = qe_m[li, g, qi].T
            idx = idxs[g][:CAP]
            de_c[b, :, :, :len(idx)] = doc_embed[:, g, idx].transpose(0, 2, 1)
            ovi = idxs[g][CAP:]
            if len(ovi):
                for li in range(L):
                    c0 = li * OV + seg_off[b]
                    dov_c[:, c0:c0 + len(ovi)] = doc_embed[li, g, ovi].T
        m = {"qe": qe_c, "de": de_c, "rt": rts}
        if OV:
            m["dov"] = dov_c
        in_maps.append(m)

    return in_maps, assign, idxs, pads_c, seg_os, qidxs, qpad


def kernel(query_embed, doc_embed, query_tok, doc_tok, r):
    in_maps, assign, idxs, pads_c, seg_os, qidxs, qpad = _stage_inputs(
        query_embed, doc_embed, query_tok, doc_tok, r)
    OV = sum(seg_os)
    seg_off = [sum(seg_os[:s]) for s in range(BPC)]

    key = (pads_c, qpad, seg_os)
    if key not in _BUILD_CACHE:
        _BUILD_CACHE[key] = _build(pads_c, qpad, seg_os)
    nc = _BUILD_CACHE[key]

    res = run_bass_kernel_spmd(nc, in_maps, core_ids=list(range(CORES)))

    out = np.zeros((BAT, L, A, BDOC), dtype=np.float32)
    for c in range(CORES):
        o_c = res.results[c]["out"]  # [BPC, L, qpad, pad_cmax]
        o2_c = res.results[c].get("out2")
        for b in range(BPC):
            g = assign[c, b]
            idx = idxs[g][:CAP]
            qi = qidxs[g]
            for li in range(L):
                out[g, li][np.ix_(qi, idx)] = o_c[b, li, :len(qi), :len(idx)]
            ovi = idxs[g][CAP:]
            if len(ovi):
                for li in range(L):
                    c0 = li * OV + seg_off[b]
                    out[g, li][np.ix_(qi, ovi)] = \
                        o2_c[:len(qi), c0:c0 + len(ovi)]
    return out


# revision 36
# speedup vs baseline: 1.1740x; 1.0075x over previous
"""LSH similarity-matrix kernel for Trainium2 (8 NeuronCores, data-parallel over batch).

Math: reference computes, per (l, b):
    c1 = (query_embed @ r.T > 0),  c2 = (doc_embed @ r.T > 0)   in {0,1}
    ham = s1 + s2 - 2*c1@c2.T ;  sim = cos(pi/NB * ham), masked where tok==0.
With +-1 codes U = 2c-1 and S = U1 @ U2.T:  ham = (NB - S)/2, so
    sim = sin(pi/(2*NB) * S).
Masks fold into the embeddings: a zeroed embedding row projects to 0,
sign(0) = 0 gives a zero code row, so S = 0 and sin(0) = 0 — exactly the
masked output. Masked doc tokens (half of them: tok in {0,1}) are gathered
away host-side entirely; output columns scatter back as zeros.

Sharding: batches are assigned to (core, slot) sorted by active-token
count; every slot is capped at 512 docs so each chunk's projection fits a
single PSUM bank. Docs beyond 512 (a few tens per heavy batch) form one
extra "overflow" job per core: fixed per-slot segments, both layers side
by side, so the SPMD program shape is identical on every core.

Precision: the projection runs as a single float32r (TF32) matmul per
128-bit chunk. TF32's 11-bit mantissa flips ~1.4k of the 71M hash bits
(those whose fp32 projection sits within rounding error of zero), which
perturbs the final similarity by rel err ~7e-3 end-to-end — well inside
the 2e-2 gate — at 1/3 the PE cost of a compensated projection. The
embeddings and r stream from HBM straight into float32r tiles (f32r is
an engine compute mode over fp32 bits, so the DMA is a plain byte copy
and no on-device cast is needed). The code dot runs as fp8e4m3 DoubleRow
matmuls (chunk pairs give K=256 per MM at 0.5 cycles/row); +-1/0 codes
and their fp32 PSUM accumulation are exact.

The kernel is sign-throughput-bound: every projected bit crosses
PSUM->SBUF through DVE/ACT exactly once. So chunks are projected in
PAIRS into one 2-bank PSUM tile and signed by a single instruction
(halving per-instruction access overhead); pairs alternate between the
DVE (clamp) and ACT (Sign) engines, weighted so both engines carry equal
ns; query pairs interleave into the first two jobs' slack. r is
pre-scaled by 2^66 host-side so the DVE clamp(x,-1,1) = max(min(x,1),-1)
sign is exact (any |proj| > 2^-66 maps to +-1). DMAs ride independent
queues (loads: SP/HWDGE, stores: Pool SWDGE, which also bypasses the
shared HWDGE dispatcher for the first doc load) so a store waiting on
Sin never blocks the next job's load.
"""
import os
import sys

sys.path.insert(0, "/opt/trn_rl_repo")

from contextlib import ExitStack

import numpy as np

import concourse.bass as bass
import concourse.mybir as mybir
import concourse.tile as tile
from concourse import bacc
from concourse.bass_utils import run_bass_kernel_spmd

L, BAT, A, BDOC, D, NB = 2, 32, 64, 1024, 128, 1024
CORES = 8
BPC = BAT // CORES          # batch slots per core
CH = NB // 128              # 8 bit-chunks
NPAIR = CH // 2             # chunk pairs
CAP = 512                   # per-slot doc cap (one PSUM bank)
SCALE = float(2.0 ** 66)
PI = float(np.pi)

F32 = mybir.dt.float32
F32R = mybir.dt.float32r
BF16 = mybir.dt.bfloat16
FP8 = mybir.dt.float8e4
Alu = mybir.AluOpType
Act = mybir.ActivationFunctionType
DR = mybir.MatmulPerfMode.DoubleRow

_BUILD_CACHE: dict = {}

# chunk-pair sign engine: 1 = DVE clamp, 0 = ACT Sign (ACT also runs Sin).
# job 6 gives DVE an extra pair: ACT (which also runs the Sins) otherwise
# finishes ~2us after DVE.
_PAIR_DVE = (1, 0, 1, 0)
_PAIR_DVE_J6 = (1, 0, 1, 1)
_QPAIR_DVE = (1, 0, 1, 1)   # 3 DVE / 1 ACT balances ACT's Sin load


def _build(pads_c: tuple, qpad: int, seg_os: tuple, reps: int = 1):
    """Per-core SPMD program. pads_c[b] <= CAP: width of main slot b;
    seg_os[b]: overflow segment width of slot b (0 = none). reps > 1
    re-emits the whole body (timing instrumentation only)."""
    pads_c = tuple(int(p) for p in pads_c)
    seg_os = tuple(int(s) for s in seg_os)
    pad_cmax = max(pads_c)
    OV = sum(seg_os)
    seg_off = [sum(seg_os[:s]) for s in range(BPC)]
    OV2 = 2 * OV                    # both layers side by side
    assert OV2 <= 512, "overflow exceeds one PSUM bank"

    nc = bacc.Bacc("TRN2", target_bir_lowering=False, debug=False)

    QW = BPC * L * qpad
    QE = nc.dram_tensor("qe", [D, QW], F32R, kind="ExternalInput").ap()
    DE = nc.dram_tensor("de", [BPC, L, D, pad_cmax], F32R,
                        kind="ExternalInput").ap()
    RT = nc.dram_tensor("rt", [D, NB], F32R, kind="ExternalInput").ap()
    OUT = nc.dram_tensor("out", [BPC, L, qpad, pad_cmax], F32,
                         kind="ExternalOutput").ap()
    if OV:
        DOV = nc.dram_tensor("dov", [D, OV2], F32R, kind="ExternalInput").ap()
        OUT2 = nc.dram_tensor("out2", [qpad, OV2], F32,
                              kind="ExternalOutput").ap()

    with tile.TileContext(nc) as tc, ExitStack() as ctx:
        const = ctx.enter_context(tc.tile_pool(name="const", bufs=1))
        jobp = ctx.enter_context(tc.tile_pool(name="jobp", bufs=4))
        outp = ctx.enter_context(tc.tile_pool(name="outp", bufs=4))
        # PSUM: chunk-pair tiles [128, 1024] (2 banks) x 3 bufs, plus the
        # dot-output tiles [*, 512] (1 bank) x 2 bufs = all 8 banks.
        ps_p = ctx.enter_context(tc.tile_pool(name="ps_p", bufs=3, space="PSUM"))

        for _rep in range(reps):
            _rp = f"r{_rep}_"
            # ---- constants, ordered for the serialized DMA-transfer queue:
            # SP/HWDGE carries the rt pieces (chunk 0-1 weights first so the
            # first projection unblocks earliest); the Pool SWDGE path
            # (bypasses the shared HWDGE dispatcher) carries the first doc
            # load + qe ----
            rt = const.tile([D, NB], F32R, tag="rt", name=f"{_rp}rt")
            nc.sync.dma_start(out=rt[:, 0:256], in_=RT[:, 0:256])
            nc.sync.dma_start(out=rt[:, 256:512], in_=RT[:, 256:512])
            qnat = const.tile([D, QW], F32R, tag="qnat", name=f"{_rp}qnat")

            # PE pre-warm: dependency-free dummy matmuls run while the first
            # DMAs land their completion receipts, pulling the PE through its
            # cold/mid clock ramp so the real projections start at 2.4 GHz.
            # warm's memset rides the (idle until ~4.5us) DVE so the Pool
            # engine can start generating the first doc load immediately.
            warm = const.tile([D, 512], BF16, tag="warm", name=f"{_rp}warm")
            nc.vector.memset(warm, 0.0)
            wps = ps_p.tile([D, 1024], F32, tag="pp",
                            name=f"{_rp}wps")[:, 0:512]
            for i in range(6):
                nc.tensor.matmul(wps, warm[:, 0:128], warm,
                                 start=True, stop=True)

            def load_consts_tail():
                nc.sync.dma_start(out=qnat, in_=QE)
                nc.sync.dma_start(out=rt[:, 512:NB], in_=RT[:, 512:NB])

            U1 = const.tile([D, CH * QW], FP8, tag="U1", name=f"{_rp}U1")

            def _pair_sign(pp, dst2, wcols, dve):
                """One instruction signs both chunks of a pair: pp cols
                [0:w] and [512:512+w] -> dst2 [p, 2, w]."""
                if wcols == 512:
                    sv = pp[:]                                  # [p, 1024]
                    dv = dst2
                else:
                    sv = pp[:].rearrange("p (h c) -> p h c",
                                         h=2)[:, :, 0:wcols]
                    dv = dst2.rearrange("p (h c) -> p h c", h=2)
                if dve:
                    nc.vector.tensor_scalar(dv, sv, 1.0, -1.0,
                                            Alu.min, Alu.max)
                else:
                    nc.scalar.activation(dv, sv, Act.Sign)

            def query_grp(g):
                # query chunk pairs 2g, 2g+1; emitted inside the first two
                # jobs so the sign work fills both engines' slack
                for pr in (2 * g, 2 * g + 1):
                    qp = ps_p.tile([D, 1024], F32, tag="pp",
                                   name=f"{_rp}qp{pr}")
                    for h in (0, 1):
                        k = 2 * pr + h
                        nc.tensor.matmul(qp[:, h * 512:h * 512 + QW],
                                         rt[:, k * 128:(k + 1) * 128], qnat,
                                         start=True, stop=True)
                    _pair_sign(qp, U1[:, 2 * pr * QW:(2 * pr + 2) * QW],
                               QW, _QPAIR_DVE[pr])

            # ---- doc jobs, software-pipelined emission ----
            jobs = [(b, l) for b in range(BPC) for l in range(L)]
            n = len(jobs)
            st = [dict() for _ in range(n + 1)]     # [-1] = overflow job

            def stage_a(j):
                b, l = jobs[j]
                pad_c = pads_c[b]
                dnat = jobp.tile([D, pad_cmax], F32R, tag="dnat",
                                 name=f"{_rp}dnat{j}")[:, 0:pad_c]
                # job 0's load takes the Pool SWDGE path: it runs
                # concurrently with the rt pieces on SP
                eng = nc.gpsimd if j == 0 else nc.sync
                eng.dma_start(out=dnat, in_=DE[b, l, :, 0:pad_c])
                st[j]["e"] = dnat

            def stage_b(j, prs=None):
                b, l = jobs[j]
                pad_c = pads_c[b]
                ev = st[j]["e"]
                if prs is None or prs[0] == 0:
                    st[j]["U2"] = jobp.tile([D, CH * pad_cmax], FP8, tag="U2",
                                            name=f"{_rp}U2{j}")
                U2 = st[j]["U2"]
                for pr in (range(NPAIR) if prs is None else prs):
                    pp = ps_p.tile([D, 1024], F32, tag="pp",
                                   name=f"{_rp}pp{j}_{pr}")
                    for h in (0, 1):
                        k = 2 * pr + h
                        nc.tensor.matmul(pp[:, h * 512:h * 512 + pad_c],
                                         rt[:, k * 128:(k + 1) * 128], ev,
                                         start=True, stop=True)
                    _pair_sign(pp, U2[:, 2 * pr * pad_c:(2 * pr + 2) * pad_c],
                               pad_c,
                               (_PAIR_DVE_J6 if j == 6 else _PAIR_DVE)[pr])

            def _dot(S, U2, pad_c, qcol, c0, c1, p0):
                for jj in range(NPAIR):
                    lw = U1[:, 2 * jj * QW:(2 * jj + 2) * QW] \
                        .rearrange("p (o c) -> p o c", o=2)[:, :, qcol:qcol + qpad]
                    rv = U2[:, 2 * jj * pad_c:(2 * jj + 2) * pad_c] \
                        .rearrange("p (o c) -> p o c", o=2)[:, :, c0:c1]
                    nc.tensor.matmul(S[:, p0:p0 + c1 - c0], lw, rv,
                                     start=(jj == 0), stop=(jj == NPAIR - 1),
                                     perf_mode=DR)

            def stage_c(j, tail=False):
                b, l = jobs[j]
                pad_c = pads_c[b]
                U2 = st[j]["U2"]
                qcol = (b * L + l) * qpad
                sim = outp.tile([qpad, pad_cmax], F32, tag="sim",
                                name=f"{_rp}sim{j}")[:, 0:pad_c]
                if tail:
                    # last job (no overflow): per-half dot->sin->store with
                    # separate S tiles so the final store isn't serialized
                    # behind the full-width sin
                    hw2 = pad_c // 2
                    for pi, (c0, c1) in enumerate([(0, hw2), (hw2, pad_c)]):
                        S = ps_p.tile([qpad, 512], F32, tag="s",
                                      bufs=2, name=f"{_rp}St{pi}")
                        _dot(S, U2, pad_c, qcol, c0, c1, 0)
                        nc.scalar.activation(sim[:, c0:c1], S[:, 0:c1 - c0],
                                             Act.Sin, scale=PI / (2.0 * NB))
                        eng = nc.sync if pi else nc.gpsimd
                        eng.dma_start(out=OUT[b, l, :, c0:c1],
                                      in_=sim[:, c0:c1])
                    return
                S = ps_p.tile([qpad, 512], F32, tag="s", bufs=2,
                              name=f"{_rp}S{j}")
                _dot(S, U2, pad_c, qcol, 0, pad_c, 0)
                nc.scalar.activation(sim, S[:, 0:pad_c], Act.Sin,
                                     scale=PI / (2.0 * NB))
                # stores ride the Pool SWDGE queue (never block loads); the
                # final job's store takes SP's lower-latency hwdge path
                eng = nc.sync if j == n - 1 else nc.gpsimd
                eng.dma_start(out=OUT[b, l, :, 0:pad_c], in_=sim)

            # ---- overflow job: one combined-layer job; segment s holds the
            # docs of slot s beyond CAP, layers side by side ----
            def stage_a_ov():
                dnat = jobp.tile([D, pad_cmax], F32R, tag="dnat",
                                 name=f"{_rp}dnatov")[:, 0:OV2]
                nc.sync.dma_start(out=dnat, in_=DOV)
                st[n]["e"] = dnat

            def stage_b_ov():
                ev = st[n]["e"]
                U2 = jobp.tile([D, CH * pad_cmax], FP8, tag="U2",
                               name=f"{_rp}U2ov")
                for pr in range(NPAIR):
                    pp = ps_p.tile([D, 1024], F32, tag="pp",
                                   name=f"{_rp}ppov{pr}")
                    for h in (0, 1):
                        k = 2 * pr + h
                        nc.tensor.matmul(pp[:, h * 512:h * 512 + OV2],
                                         rt[:, k * 128:(k + 1) * 128], ev,
                                         start=True, stop=True)
                    _pair_sign(pp, U2[:, 2 * pr * OV2:(2 * pr + 2) * OV2],
                               OV2, _PAIR_DVE[pr])
                st[n]["U2"] = U2

            def stage_c_ov():
                U2 = st[n]["U2"]
                S = ps_p.tile([qpad, 512], F32, tag="s", bufs=2,
                              name=f"{_rp}Sov")
                sim = outp.tile([qpad, pad_cmax], F32, tag="sim",
                                name=f"{_rp}simov")[:, 0:OV2]
                for li in range(L):
                    for s in range(BPC):
                        if not seg_os[s]:
                            continue
                        c0 = li * OV + seg_off[s]
                        qcol = (s * L + li) * qpad
                        _dot(S, U2, OV2, qcol, c0, c0 + seg_os[s], c0)
                nc.scalar.activation(sim, S[:, 0:OV2], Act.Sin,
                                     scale=PI / (2.0 * NB))
                nc.sync.dma_start(out=OUT2, in_=sim)

            stage_a(0)
            load_consts_tail()
            # job 0's pairs interleave with the query pairs so both sign
            # engines run continuously from the first DMA landing: pairs
            # 0-1 + query chunks 0-3 need only rt[:512]+qe; the rest rt[512:]
            stage_b(0, prs=(0, 1))
            query_grp(0)
            stage_b(0, prs=(2, 3))
            query_grp(1)
            stage_a(1)
            stage_a(2)
            stage_b(1)
            stage_a(3)
            for j in range(n):
                if OV and j == n - 3:
                    stage_b_ov()
                if j + 2 < n:
                    stage_b(j + 2)
                stage_c(j, tail=(not OV and j == n - 1))
                if j + 4 < n:
                    stage_a(j + 4)
                elif OV and j + 4 == n:
                    stage_a_ov()
            if OV:
                stage_c_ov()

    nc.compile()
    return nc


def _stage_inputs(query_embed, doc_embed, query_tok, doc_tok, r):
    query_embed = np.ascontiguousarray(query_embed, dtype=np.float32)
    doc_embed = np.ascontiguousarray(doc_embed, dtype=np.float32)
    r = np.ascontiguousarray(r, dtype=np.float32)

    qmask = (np.asarray(query_tok) != 0)
    dmask = (np.asarray(doc_tok) != 0)

    # sort batches by active count; slot s takes ranks [s*CORES, (s+1)*CORES)
    # spread across the 8 cores, so per-slot padding is tight and identical
    # on every core (SPMD requires one shape per slot)
    counts = dmask.sum(axis=1).astype(int)
    order = np.argsort(counts, kind="stable")
    assign = np.empty((CORES, BPC), dtype=int)   # assign[c, b] = batch id
    for s in range(BPC):
        for c in range(CORES):
            assign[c, s] = order[s * CORES + c]
    maxes = [int(counts[assign[:, s]].max()) for s in range(BPC)]
    pads_c = tuple(min(CAP, max(64, -(-m // 32) * 32)) for m in maxes)
    seg_os = tuple(-(-max(0, m - CAP) // 16) * 16 for m in maxes)
    pad_cmax = max(pads_c)
    OV = sum(seg_os)
    seg_off = [sum(seg_os[:s]) for s in range(BPC)]

    qe_m = query_embed * qmask[None, :, :, None].astype(np.float32)
    qidxs = [np.flatnonzero(qmask[g]) for g in range(BAT)]
    qpad = min(A, max(16, int(-(-max(len(q) for q in qidxs) // 16) * 16)))
    QW = BPC * L * qpad

    rts = np.ascontiguousarray(r.T * SCALE)          # [D, NB], fp32 bits

    idxs = [np.flatnonzero(dmask[g]) for g in range(BAT)]
    in_maps = []
    for c in range(CORES):
        # embeddings staged pre-transposed [D, tokens]; queries compacted
        # to their active rows (masks are per-batch, shared by both layers)
        qe_c = np.zeros((D, QW), dtype=np.float32)
        de_c = np.zeros((BPC, L, D, pad_cmax), dtype=np.float32)
        dov_c = np.zeros((D, 2 * OV), dtype=np.float32)
        for b in range(BPC):
            g = assign[c, b]
            qi = qidxs[g]
            for li in range(L):
                col = (b * L + li) * qpad
                qe_c[:, col:col + len(qi)] = qe_m[li, g, qi].T
            idx = idxs[g][:CAP]
            de_c[b, :, :, :len(idx)] = doc_embed[:, g, idx].transpose(0, 2, 1)
            ovi = idxs[g][CAP:]
            if len(ovi):
                for li in range(L):
                    c0 = li * OV + seg_off[b]
                    dov_c[:, c0:c0 + len(ovi)] = doc_embed[li, g, ovi].T
        m = {"qe": qe_c, "de": de_c, "rt": rts}
        if OV:
            m["dov"] = dov_c
        in_maps.append(m)

    return in_maps, assign, idxs, pads_c, seg_os, qidxs, qpad


def kernel(query_embed, doc_embed, query_tok, doc_tok, r):
    in_maps, assign, idxs, pads_c, seg_os, qidxs, qpad = _stage_inputs(
        query_embed, doc_embed, query_tok, doc_tok, r)
    OV = sum(seg_os)
    seg_off = [sum(seg_os[:s]) for s in range(BPC)]

    key = (pads_c, qpad, seg_os)
    if key not in _BUILD_CACHE:
        _BUILD_CACHE[key] = _build(pads_c, qpad, seg_os)
    nc = _BUILD_CACHE[key]

    res = run_bass_kernel_spmd(nc, in_maps, core_ids=list(range(CORES)))

    out = np.zeros((BAT, L, A, BDOC), dtype=np.float32)
    for c in range(CORES):
        o_c = res.results[c]["out"]  # [BPC, L, qpad, pad_cmax]
        o2_c = res.results[c].get("out2")
        for b in range(BPC):
            g = assign[c, b]
            idx = idxs[g][:CAP]
            qi = qidxs[g]
            for li in range(L):
                out[g, li][np.ix_(qi, idx)] = o_c[b, li, :len(qi), :len(idx)]
            ovi = idxs[g][CAP:]
            if len(ovi):
                for li in range(L):
                    c0 = li * OV + seg_off[b]
                    out[g, li][np.ix_(qi, ovi)] = \
                        o2_c[:len(qi), c0:c0 + len(ovi)]
    return out


# revision 45
# speedup vs baseline: 1.2663x; 1.0787x over previous
"""LSH similarity-matrix kernel for Trainium2 (8 NeuronCores, data-parallel over batch).

Math: reference computes, per (l, b):
    c1 = (query_embed @ r.T > 0),  c2 = (doc_embed @ r.T > 0)   in {0,1}
    ham = s1 + s2 - 2*c1@c2.T ;  sim = cos(pi/NB * ham), masked where tok==0.
With +-1 codes U = 2c-1 and S = U1 @ U2.T:  ham = (NB - S)/2, so
    sim = sin(pi/(2*NB) * S).
Masks fold into the embeddings: a zeroed embedding row projects to 0,
sign(0) = 0 gives a zero code row, so S = 0 and sin(0) = 0 — exactly the
masked output. Masked doc tokens (half of them: tok in {0,1}) are gathered
away host-side entirely; output columns scatter back as zeros.

Sharding: batches are assigned to (core, slot) sorted by active-token
count; every slot is capped at 512 docs so each chunk's projection fits a
single PSUM bank. Docs beyond 512 (a few tens per heavy batch) ride in
fixed per-slot segments appended to the query columns of the shared
"aux" tile (query width + 2*overflow <= 512), so the SPMD program shape
is identical on every core and the overflow costs no extra projection,
sign instructions, or jobs — only a small extra dot+sin+store that runs
early, far off the critical tail.

Precision: the projection runs as a single float32r (TF32) matmul per
128-bit chunk. TF32's 11-bit mantissa flips ~1.4k of the 71M hash bits
(those whose fp32 projection sits within rounding error of zero), which
perturbs the final similarity by rel err ~7e-3 end-to-end — well inside
the 2e-2 gate — at 1/3 the PE cost of a compensated projection. The
embeddings and r stream from HBM straight into float32r tiles (f32r is
an engine compute mode over fp32 bits, so the DMA is a plain byte copy
and no on-device cast is needed). The code dot runs as fp8e4m3 DoubleRow
matmuls (chunk pairs give K=256 per MM at 0.5 cycles/row); +-1/0 codes
and their fp32 PSUM accumulation are exact.

The kernel is sign-throughput-bound: every projected bit crosses
PSUM->SBUF through DVE/ACT exactly once. So chunks are projected in
PAIRS into one 2-bank PSUM tile and signed by a single instruction
(halving per-instruction access overhead); pairs alternate between the
DVE (clamp) and ACT (Sign) engines, weighted so both engines carry equal
ns; aux (query+overflow) pairs interleave into the first jobs' slack. r is
pre-scaled by 2^66 host-side so the DVE clamp(x,-1,1) = max(min(x,1),-1)
sign is exact (any |proj| > 2^-66 maps to +-1). DMAs ride independent
queues (loads: SP/HWDGE, stores: Pool SWDGE, which also bypasses the
shared HWDGE dispatcher for the first doc load) so a store waiting on
Sin never blocks the next job's load.
"""
import os
import sys

sys.path.insert(0, "/opt/trn_rl_repo")

from contextlib import ExitStack

import numpy as np

import concourse.bass as bass
import concourse.mybir as mybir
import concourse.tile as tile
from concourse import bacc
from concourse.bass_utils import run_bass_kernel_spmd

L, BAT, A, BDOC, D, NB = 2, 32, 64, 1024, 128, 1024
CORES = 8
BPC = BAT // CORES          # batch slots per core
CH = NB // 128              # 8 bit-chunks
NPAIR = CH // 2             # chunk pairs
CAP = 512                   # per-slot doc cap (one PSUM bank)
SCALE = float(2.0 ** 66)
PI = float(np.pi)

F32 = mybir.dt.float32
F32R = mybir.dt.float32r
BF16 = mybir.dt.bfloat16
FP8 = mybir.dt.float8e4
Alu = mybir.AluOpType
Act = mybir.ActivationFunctionType
DR = mybir.MatmulPerfMode.DoubleRow

_BUILD_CACHE: dict = {}

# chunk-pair sign engine: 1 = DVE clamp, 0 = ACT Sign (ACT also runs Sin).
_PAIR_DVE = (1, 0, 1, 0)
_QPAIR_DVE = (0, 1, 1, 0)   # 2/2 split balances with the wider aux instrs


def _build(pads_c: tuple, qpad: int, seg_os: tuple, reps: int = 1):
    """Per-core SPMD program. pads_c[b] <= CAP: width of main slot b;
    seg_os[b]: overflow segment width of slot b (0 = none). reps > 1
    re-emits the whole body (timing instrumentation only)."""
    pads_c = tuple(int(p) for p in pads_c)
    seg_os = tuple(int(s) for s in seg_os)
    pad_cmax = max(pads_c)
    OV = sum(seg_os)
    seg_off = [sum(seg_os[:s]) for s in range(BPC)]
    OV2 = 2 * OV                    # both layers side by side
    assert OV2 <= 512, "overflow exceeds one PSUM bank"

    nc = bacc.Bacc("TRN2", target_bir_lowering=False, debug=False)

    QW = BPC * L * qpad
    W = QW + OV2                    # query columns + overflow doc columns
    assert W <= 512, "aux tile exceeds one PSUM bank"
    QE = nc.dram_tensor("qe", [D, W], F32R, kind="ExternalInput").ap()
    DE = nc.dram_tensor("de", [BPC, L, D, pad_cmax], F32R,
                        kind="ExternalInput").ap()
    RT = nc.dram_tensor("rt", [D, NB], F32R, kind="ExternalInput").ap()
    OUT = nc.dram_tensor("out", [BPC, L, qpad, pad_cmax], F32,
                         kind="ExternalOutput").ap()
    if OV:
        OUT2 = nc.dram_tensor("out2", [qpad, OV2], F32,
                              kind="ExternalOutput").ap()

    with tile.TileContext(nc) as tc, ExitStack() as ctx:
        const = ctx.enter_context(tc.tile_pool(name="const", bufs=1))
        jobp = ctx.enter_context(tc.tile_pool(name="jobp", bufs=4))
        outp = ctx.enter_context(tc.tile_pool(name="outp", bufs=4))
        # PSUM: chunk-pair tiles [128, 1024] (2 banks) x 3 bufs, plus the
        # dot-output tiles [*, 512] (1 bank) x 2 bufs = all 8 banks.
        ps_p = ctx.enter_context(tc.tile_pool(name="ps_p", bufs=3, space="PSUM"))

        for _rep in range(reps):
            _rp = f"r{_rep}_"
            # ---- constants, ordered for the serialized DMA-transfer queue:
            # SP/HWDGE carries the rt pieces (chunk 0-1 weights first so the
            # first projection unblocks earliest); the Pool SWDGE path
            # (bypasses the shared HWDGE dispatcher) carries the first doc
            # load + qe ----
            rt = const.tile([D, NB], F32R, tag="rt", name=f"{_rp}rt")
            nc.sync.dma_start(out=rt[:, 0:256], in_=RT[:, 0:256])
            nc.sync.dma_start(out=rt[:, 256:512], in_=RT[:, 256:512])
            qnat = const.tile([D, W], F32R, tag="qnat", name=f"{_rp}qnat")

            # PE pre-warm: dependency-free dummy matmuls run while the first
            # DMAs land their completion receipts, pulling the PE through its
            # cold/mid clock ramp so the real projections start at 2.4 GHz.
            # warm's memset rides the (idle until ~4.5us) DVE so the Pool
            # engine can start generating the first doc load immediately.
            warm = const.tile([D, 512], BF16, tag="warm", name=f"{_rp}warm")
            nc.vector.memset(warm, 0.0)
            wps = ps_p.tile([D, 1024], F32, tag="pp",
                            name=f"{_rp}wps")[:, 0:512]
            for i in range(4):
                nc.tensor.matmul(wps, warm[:, 0:128], warm,
                                 start=True, stop=True)

            def load_consts_tail():
                nc.sync.dma_start(out=qnat, in_=QE)
                nc.sync.dma_start(out=rt[:, 512:768], in_=RT[:, 512:768])
                nc.sync.dma_start(out=rt[:, 768:NB], in_=RT[:, 768:NB])

            U1 = const.tile([D, CH * W], FP8, tag="U1", name=f"{_rp}U1")

            def _pair_sign(pp, dst2, wcols, dve):
                """One instruction signs both chunks of a pair: pp cols
                [0:w] and [512:512+w] -> dst2 [p, 2, w]."""
                if wcols == 512:
                    sv = pp[:]                                  # [p, 1024]
                    dv = dst2
                else:
                    sv = pp[:].rearrange("p (h c) -> p h c",
                                         h=2)[:, :, 0:wcols]
                    dv = dst2.rearrange("p (h c) -> p h c", h=2)
                if dve:
                    nc.vector.tensor_scalar(dv, sv, 1.0, -1.0,
                                            Alu.min, Alu.max)
                else:
                    nc.scalar.activation(dv, sv, Act.Sign)

            def query_grp(g):
                # aux chunk pairs 2g, 2g+1 (query + overflow-doc columns in
                # one moving tile); emitted inside the first two jobs so the
                # sign work fills both engines' startup slack
                for pr in (2 * g, 2 * g + 1):
                    qp = ps_p.tile([D, 1024], F32, tag="pp",
                                   name=f"{_rp}qp{pr}")
                    for h in (0, 1):
                        k = 2 * pr + h
                        nc.tensor.matmul(qp[:, h * 512:h * 512 + W],
                                         rt[:, k * 128:(k + 1) * 128], qnat,
                                         start=True, stop=True)
                    _pair_sign(qp, U1[:, 2 * pr * W:(2 * pr + 2) * W],
                               W, _QPAIR_DVE[pr])

            # ---- doc jobs, software-pipelined emission ----
            jobs = [(b, l) for b in range(BPC) for l in range(L)]
            n = len(jobs)
            st = [dict() for _ in range(n + 1)]     # [-1] = overflow job

            def stage_a(j):
                b, l = jobs[j]
                pad_c = pads_c[b]
                dnat = jobp.tile([D, pad_cmax], F32R, tag="dnat",
                                 name=f"{_rp}dnat{j}")[:, 0:pad_c]
                # job 0's load takes the Pool SWDGE path: it runs
                # concurrently with the rt pieces on SP
                eng = nc.gpsimd if j == 0 else nc.sync
                eng.dma_start(out=dnat, in_=DE[b, l, :, 0:pad_c])
                st[j]["e"] = dnat

            def stage_b(j, prs=None):
                b, l = jobs[j]
                pad_c = pads_c[b]
                ev = st[j]["e"]
                if prs is None or prs[0] == 0:
                    st[j]["U2"] = jobp.tile([D, CH * pad_cmax], FP8, tag="U2",
                                            name=f"{_rp}U2{j}")
                U2 = st[j]["U2"]
                for pr in (range(NPAIR) if prs is None else prs):
                    pp = ps_p.tile([D, 1024], F32, tag="pp",
                                   name=f"{_rp}pp{j}_{pr}")
                    for h in (0, 1):
                        k = 2 * pr + h
                        nc.tensor.matmul(pp[:, h * 512:h * 512 + pad_c],
                                         rt[:, k * 128:(k + 1) * 128], ev,
                                         start=True, stop=True)
                    _pair_sign(pp, U2[:, 2 * pr * pad_c:(2 * pr + 2) * pad_c],
                               pad_c, _PAIR_DVE[pr])

            def _dot(S, U2, pad_c, qcol, c0, c1, p0):
                for jj in range(NPAIR):
                    lw = U1[:, 2 * jj * W:(2 * jj + 2) * W] \
                        .rearrange("p (o c) -> p o c", o=2)[:, :, qcol:qcol + qpad]
                    rv = U2[:, 2 * jj * pad_c:(2 * jj + 2) * pad_c] \
                        .rearrange("p (o c) -> p o c", o=2)[:, :, c0:c1]
                    nc.tensor.matmul(S[:, p0:p0 + c1 - c0], lw, rv,
                                     start=(jj == 0), stop=(jj == NPAIR - 1),
                                     perf_mode=DR)

            def stage_c(j, tail=False):
                b, l = jobs[j]
                pad_c = pads_c[b]
                U2 = st[j]["U2"]
                qcol = (b * L + l) * qpad
                sim = outp.tile([qpad, pad_cmax], F32, tag="sim",
                                name=f"{_rp}sim{j}")[:, 0:pad_c]
                if tail:
                    # last job (no overflow): per-half dot->sin->store with
                    # separate S tiles so the final store isn't serialized
                    # behind the full-width sin
                    hw2 = pad_c // 2
                    for pi, (c0, c1) in enumerate([(0, hw2), (hw2, pad_c)]):
                        S = ps_p.tile([qpad, 512], F32, tag="s",
                                      bufs=2, name=f"{_rp}St{pi}")
                        _dot(S, U2, pad_c, qcol, c0, c1, 0)
                        nc.scalar.activation(sim[:, c0:c1], S[:, 0:c1 - c0],
                                             Act.Sin, scale=PI / (2.0 * NB))
                        eng = nc.sync if pi else nc.gpsimd
                        eng.dma_start(out=OUT[b, l, :, c0:c1],
                                      in_=sim[:, c0:c1])
                    return
                S = ps_p.tile([qpad, 512], F32, tag="s", bufs=2,
                              name=f"{_rp}S{j}")
                _dot(S, U2, pad_c, qcol, 0, pad_c, 0)
                nc.scalar.activation(sim, S[:, 0:pad_c], Act.Sin,
                                     scale=PI / (2.0 * NB))
                # stores ride the Pool SWDGE queue (never block loads); the
                # final job's store takes SP's lower-latency hwdge path
                eng = nc.sync if j == n - 1 else nc.gpsimd
                eng.dma_start(out=OUT[b, l, :, 0:pad_c], in_=sim)

            # ---- overflow docs live in the aux tile's columns [QW:W]:
            # their codes are produced by the aux pair signs, so only the
            # dot+sin+store remain — emitted early, right after c(0), far
            # off the tail ----
            def stage_c_ov():
                S = ps_p.tile([qpad, 512], F32, tag="s", bufs=2,
                              name=f"{_rp}Sov")
                sim = outp.tile([qpad, pad_cmax], F32, tag="sim",
                                name=f"{_rp}simov")[:, 0:OV2]
                for li in range(L):
                    for s in range(BPC):
                        if not seg_os[s]:
                            continue
                        c0 = li * OV + seg_off[s]
                        qcol = (s * L + li) * qpad
                        _dot(S, U1, W, qcol, QW + c0, QW + c0 + seg_os[s], c0)
                nc.scalar.activation(sim, S[:, 0:OV2], Act.Sin,
                                     scale=PI / (2.0 * NB))
                nc.gpsimd.dma_start(out=OUT2, in_=sim)

            stage_a(0)
            load_consts_tail()
            # job 0's pairs interleave with the query pairs so both sign
            # engines run continuously from the first DMA landing: pairs
            # 0-1 + query chunks 0-3 need only rt[:512]+qe; the rest rt[512:]
            stage_b(0, prs=(0, 1))
            query_grp(0)
            stage_b(0, prs=(2, 3))
            query_grp(1)
            stage_a(1)
            stage_a(2)
            stage_b(1)
            stage_a(3)
            for j in range(n):
                if j + 2 < n:
                    stage_b(j + 2)
                stage_c(j, tail=False)
                if OV and j == 0:
                    stage_c_ov()
                if j + 4 < n:
                    stage_a(j + 4)

    nc.compile()
    return nc


def _stage_inputs(query_embed, doc_embed, query_tok, doc_tok, r):
    query_embed = np.ascontiguousarray(query_embed, dtype=np.float32)
    doc_embed = np.ascontiguousarray(doc_embed, dtype=np.float32)
    r = np.ascontiguousarray(r, dtype=np.float32)

    qmask = (np.asarray(query_tok) != 0)
    dmask = (np.asarray(doc_tok) != 0)

    # sort batches by active count; slot s takes ranks [s*CORES, (s+1)*CORES)
    # spread across the 8 cores, so per-slot padding is tight and identical
    # on every core (SPMD requires one shape per slot)
    counts = dmask.sum(axis=1).astype(int)
    order = np.argsort(counts, kind="stable")
    assign = np.empty((CORES, BPC), dtype=int)   # assign[c, b] = batch id
    for s in range(BPC):
        for c in range(CORES):
            assign[c, s] = order[s * CORES + c]
    maxes = [int(counts[assign[:, s]].max()) for s in range(BPC)]
    pads_c = tuple(min(CAP, max(64, -(-m // 32) * 32)) for m in maxes)
    seg_os = tuple(-(-max(0, m - CAP) // 8) * 8 for m in maxes)
    pad_cmax = max(pads_c)
    OV = sum(seg_os)
    seg_off = [sum(seg_os[:s]) for s in range(BPC)]

    qe_m = query_embed * qmask[None, :, :, None].astype(np.float32)
    qidxs = [np.flatnonzero(qmask[g]) for g in range(BAT)]
    qpad = min(A, max(16, int(-(-max(len(q) for q in qidxs) // 8) * 8)))
    QW = BPC * L * qpad

    rts = np.ascontiguousarray(r.T * SCALE)          # [D, NB], fp32 bits

    idxs = [np.flatnonzero(dmask[g]) for g in range(BAT)]
    in_maps = []
    for c in range(CORES):
        # embeddings staged pre-transposed [D, tokens]; queries compacted
        # to their active rows (masks are per-batch, shared by both layers)
        # aux = compacted queries followed by the overflow-doc segments
        qe_c = np.zeros((D, QW + 2 * OV), dtype=np.float32)
        de_c = np.zeros((BPC, L, D, pad_cmax), dtype=np.float32)
        for b in range(BPC):
            g = assign[c, b]
            qi = qidxs[g]
            for li in range(L):
                col = (b * L + li) * qpad
                qe_c[:, col:col + len(qi)] = qe_m[li, g, qi].T
            idx = idxs[g][:CAP]
            de_c[b, :, :, :len(idx)] = doc_embed[:, g, idx].transpose(0, 2, 1)
            ovi = idxs[g][CAP:]
            if len(ovi):
                for li in range(L):
                    c0 = QW + li * OV + seg_off[b]
                    qe_c[:, c0:c0 + len(ovi)] = doc_embed[li, g, ovi].T
        in_maps.append({"qe": qe_c, "de": de_c, "rt": rts})

    return in_maps, assign, idxs, pads_c, seg_os, qidxs, qpad


def kernel(query_embed, doc_embed, query_tok, doc_tok, r):
    in_maps, assign, idxs, pads_c, seg_os, qidxs, qpad = _stage_inputs(
        query_embed, doc_embed, query_tok, doc_tok, r)
    OV = sum(seg_os)
    seg_off = [sum(seg_os[:s]) for s in range(BPC)]

    key = (pads_c, qpad, seg_os)
    if key not in _BUILD_CACHE:
        _BUILD_CACHE[key] = _build(pads_c, qpad, seg_os)
    nc = _BUILD_CACHE[key]

    res = run_bass_kernel_spmd(nc, in_maps, core_ids=list(range(CORES)))

    out = np.zeros((BAT, L, A, BDOC), dtype=np.float32)
    for c in range(CORES):
        o_c = res.results[c]["out"]  # [BPC, L, qpad, pad_cmax]
        o2_c = res.results[c].get("out2")
        for b in range(BPC):
            g = assign[c, b]
            idx = idxs[g][:CAP]
            qi = qidxs[g]
            for li in range(L):
                out[g, li][np.ix_(qi, idx)] = o_c[b, li, :len(qi), :len(idx)]
            ovi = idxs[g][CAP:]
            if len(ovi):
                for li in range(L):
                    c0 = li * OV + seg_off[b]
                    out[g, li][np.ix_(qi, ovi)] = \
                        o2_c[:len(qi), c0:c0 + len(ovi)]
    return out
